# revision 14
# baseline (speedup 1.0000x reference)
"""DiT block kernel for Trainium2, 8 NeuronCores.

Sharding: core ci = (batch b = ci//2, token-half h = ci%2). Each core computes
the full DiT block for 512 "q" tokens of one batch. Attention needs all 1024
k/v tokens of the batch, so the per-batch LN/qkv-kv work is duplicated across
the 2 cores sharing a batch (≈14% FLOP overhead, zero collectives).

Everything on-device runs in a "transposed" activation layout [feature, token]
so that per-feature modulation vectors (adaLN shift/scale/gate) and all biases
are per-partition scalars, and all matmuls consume weights in their natural
or host-pre-tiled layout with zero on-device transposes of big tensors.

Key layout identities (out = lhsT.T @ rhs on the PE):
  cm       [1,6H]    : lhsT = silu(c).T k-slabs,  rhs = w_mod slabs
  q/kT     [C, tok]  : lhsT = w_qkv tiles,        rhs = xmT
  v        [tok, C]  : lhsT = xmT,                rhs = w_v slabs
  scoresT  [kt, qt]  : lhsT = kT head slice,      rhs = qT head slice (K=64)
  att_avT  [65, qt]  : lhsT = [v_head | ones],    rhs = exp(scoresT)  (row 64
                       accumulates the softmax denominator for free)
  yT       [H, tok]  : lhsT = w_proj tiles,       rhs = att_outT
  hidT     [dff,tok] : lhsT = w1 tiles,           rhs = xm2T
  finalT   [H, tok]  : lhsT = w2 tiles (bf16),    rhs = hidT (bf16)

LN stats (mean / mean-of-squares over the H partition dim) are computed with
ones-vector matmuls on the PE; rstd = exp(-0.5*ln(var+eps)) on ACT (keeps the
Ln/Exp table set hot, avoids the banned Rsqrt).
"""

import sys

for _p in ("/opt/trn_rl_repo",):
    if _p not in sys.path:
        sys.path.insert(0, _p)

import numpy as np
import ml_dtypes

B, N, H = 4, 1024, 1024
NH, D = 16, 64
DFF = 4 * H
EPS = 1e-5
NCORES = 8
TOK = N // 2      # q tokens per core
KT = H // 128     # 8  H-tiles
MT_QK = 16        # q+k column tiles
MT_FF = DFF // 128  # 32
VA = NH * (D + 1)   # 1040 v_aug columns

_CACHE = {}


def _build(taps=()):
    import concourse.bacc as bacc
    import concourse.tile as tile
    import concourse.mybir as mybir
    from concourse.masks import make_identity

    f32 = mybir.dt.float32
    f32r = mybir.dt.float32r
    bf16 = mybir.dt.bfloat16
    AF = mybir.ActivationFunctionType
    OP = mybir.AluOpType

    nc = bacc.Bacc("TRN2", target_bir_lowering=False, debug=False,
                   num_devices=NCORES)

    dxT = nc.dram_tensor("xT", [H, N], f32, kind="ExternalInput")
    dcT = nc.dram_tensor("cT", [128, KT], f32, kind="ExternalInput")
    dwmod = nc.dram_tensor("wmod", [12, KT, 128, 512], f32, kind="ExternalInput")
    dbmodT = nc.dram_tensor("bmodT", [128, 48], f32, kind="ExternalInput")
    dwqk = nc.dram_tensor("wqk", [MT_QK, 128, KT, 128], f32, kind="ExternalInput")
    dbqkT = nc.dram_tensor("bqkT", [128, MT_QK], f32, kind="ExternalInput")
    dwv = nc.dram_tensor("wv", [H, H], f32, kind="ExternalInput")
    dbvT = nc.dram_tensor("bvT", [128, KT], f32, kind="ExternalInput")
    dwproj = nc.dram_tensor("wproj", [KT, 128, KT, 128], f32, kind="ExternalInput")
    dbprojT = nc.dram_tensor("bprojT", [128, KT], f32, kind="ExternalInput")
    dw1 = nc.dram_tensor("w1", [MT_FF, 128, KT, 128], f32, kind="ExternalInput")
    db1T = nc.dram_tensor("b1T", [128, MT_FF], f32, kind="ExternalInput")
    dw2 = nc.dram_tensor("w2", [KT, 128, MT_FF, 128], bf16, kind="ExternalInput")
    db2T = nc.dram_tensor("b2T", [128, KT], f32, kind="ExternalInput")
    doutT = nc.dram_tensor("outT", [H, TOK], f32, kind="ExternalOutput")

    _tapt = {}

    def tap(name, ap, shape=None, dtype=None):
        if name not in taps:
            return
        if name not in _tapt:
            _tapt[name] = nc.dram_tensor(
                "tap_" + name, list(shape or ap.shape), dtype or ap.dtype,
                kind="ExternalOutput")
        nc.sync.dma_start(out=_tapt[name].ap(), in_=ap)

    def r(ap):
        return ap.bitcast(f32r)

    with tile.TileContext(nc) as tc:
        from contextlib import ExitStack
        es = ExitStack()
        with es:
            pconst = es.enter_context(tc.tile_pool(name="const", bufs=1))
            big = es.enter_context(tc.tile_pool(name="big", bufs=1))
            pmm = es.enter_context(tc.tile_pool(name="pmm", bufs=2, space="PSUM"))
            psc = es.enter_context(tc.tile_pool(name="psc", bufs=2, space="PSUM"))
            pav = es.enter_context(tc.tile_pool(name="pav", bufs=2, space="PSUM"))

            # ---- constants ----
            ident = pconst.tile([128, 128], f32)
            make_identity(nc, ident)
            ones = pconst.tile([128, 1], f32)
            nc.vector.memset(ones[:].bitcast(mybir.dt.uint32), 0x3F800000)
            ones_bf = pconst.tile([128, 1], bf16)
            nc.vector.memset(ones_bf, 1.0)

            cT_sb = pconst.tile([128, KT], f32)
            nc.sync.dma_start(out=cT_sb, in_=dcT.ap())
            bmodT = pconst.tile([128, 48], f32)
            nc.sync.dma_start(out=bmodT, in_=dbmodT.ap())
            bqkT = pconst.tile([128, MT_QK], f32)
            nc.sync.dma_start(out=bqkT, in_=dbqkT.ap())
            bvT = pconst.tile([128, KT], f32)
            nc.sync.dma_start(out=bvT, in_=dbvT.ap())
            bprojT = pconst.tile([128, KT], f32)
            nc.sync.dma_start(out=bprojT, in_=dbprojT.ap())
            b1T = pconst.tile([128, MT_FF], f32)
            nc.sync.dma_start(out=b1T, in_=db1T.ap())
            b2T = pconst.tile([128, KT], f32)
            nc.sync.dma_start(out=b2T, in_=db2T.ap())

            silucT = pconst.tile([128, KT], f32)
            nc.scalar.activation(out=silucT[:].bitcast(f32r), in_=cT_sb, func=AF.Silu)
            eps_sb = pconst.tile([1, 1], f32)
            nc.vector.memset(eps_sb, EPS)

            # ---- xT load ----
            xT = big.tile([128, KT, N], f32, tag="A")
            for k in range(KT):
                nc.sync.dma_start(out=xT[:, k, :].bitcast(f32r),
                                  in_=dxT.ap()[k * 128:(k + 1) * 128, :].bitcast(f32r))

            # ---- cm = silu(c) @ w_mod ; staged via DRAM into [48, 128] ----
            cm_rs = pconst.tile([48, 128], f32)
            with tc.tile_pool(name="pwmod", bufs=8) as pwmod, \
                 tc.tile_pool(name="pcmb", bufs=2) as pcmb, \
                 tc.tile_pool(name="pcmd", bufs=1, space="DRAM") as pcmd:
                cmdram = pcmd.tile([12, 512], f32)
                for nchk in range(12):
                    ps = pmm.tile([1, 512], f32, tag="mm")
                    for k in range(KT):
                        wt = pwmod.tile([128, 512], f32, tag="wmod")
                        nc.sync.dma_start(out=wt[:].bitcast(f32r), in_=dwmod.ap()[nchk, k].bitcast(f32r))
                        nc.tensor.matmul(ps, r(silucT[:, k:k + 1]), r(wt[:]),
                                         start=(k == 0), stop=(k == KT - 1))
                    cb = pcmb.tile([1, 512], f32, tag="cmb")
                    nc.any.tensor_copy(out=cb, in_=ps[:])
                    nc.sync.dma_start(out=cmdram[nchk:nchk + 1, :], in_=cb[:])
                nc.sync.dma_start(
                    out=cm_rs,
                    in_=cmdram[:].rearrange("a (b f) -> (a b) f", b=4))
            tap("cm_rs", cm_rs[:])
            ps_cmT = pmm.tile([128, 48], f32, tag="mm")
            nc.tensor.transpose(ps_cmT, cm_rs[:], ident[0:48, 0:48])
            cmT = pconst.tile([128, 48], f32)
            nc.vector.tensor_add(out=cmT, in0=ps_cmT[:], in1=bmodT[:])
            tap("cmT_raw", cmT[:])
            # cmT columns: v*8+t ; v: 0 sh_msa, 1 sc_msa, 2 g_msa, 3 sh_mlp,
            # 4 sc_mlp, 5 g_mlp
            sc1p = pconst.tile([128, KT], f32)
            nc.vector.tensor_scalar_add(out=sc1p, in0=cmT[:, 8:16], scalar1=1.0)
            g1p = pconst.tile([128, KT], f32)
            nc.vector.tensor_scalar_add(out=g1p, in0=cmT[:, 16:24], scalar1=1.0)
            sc2p = pconst.tile([128, KT], f32)
            nc.vector.tensor_scalar_add(out=sc2p, in0=cmT[:, 32:40], scalar1=1.0)
            tap("cmT", cmT[:])

            # ---- LN1 stats over H (partition dim) via ones-matmuls ----
            pmub_cm = tc.tile_pool(name="pmub", bufs=1)
            pmub = pmub_cm.__enter__()
            muB = pmub.tile([128, N], f32)
            rstdB = pmub.tile([128, N], f32)
            pstat_cm = tc.tile_pool(name="pstat", bufs=1)
            pstat = pstat_cm.__enter__()
            st_mu = pstat.tile([1, N], f32)
            st_e2 = pstat.tile([1, N], f32)
            st_var = pstat.tile([1, N], f32)
            st_rstd = pstat.tile([1, N], f32)

            with tc.tile_pool(name="pxsq", bufs=2) as pxsq:
                for nch in range(2):
                    ps = pmm.tile([1, 512], f32, tag="mm")
                    for k in range(KT):
                        nc.tensor.matmul(
                            ps, r(ones[:]), r(xT[:, k, nch * 512:(nch + 1) * 512]),
                            start=(k == 0), stop=(k == KT - 1))
                    nc.scalar.activation(out=st_mu[0:1, nch * 512:(nch + 1) * 512],
                                         in_=ps[:], func=AF.Copy, scale=1.0 / H)
                for nch in range(2):
                    ps = pmm.tile([1, 512], f32, tag="mm")
                    for k in range(KT):
                        xsq = pxsq.tile([128, 512], bf16, tag="xsq")
                        nc.vector.tensor_mul(
                            out=xsq, in0=xT[:, k, nch * 512:(nch + 1) * 512],
                            in1=xT[:, k, nch * 512:(nch + 1) * 512])
                        nc.tensor.matmul(ps, ones_bf[:], xsq[:],
                                         start=(k == 0), stop=(k == KT - 1))
                    nc.scalar.activation(out=st_e2[0:1, nch * 512:(nch + 1) * 512],
                                         in_=ps[:], func=AF.Copy, scale=1.0 / H)
            nc.vector.tensor_mul(out=st_var, in0=st_mu, in1=st_mu)
            nc.vector.tensor_sub(out=st_var, in0=st_e2, in1=st_var)
            # rstd = exp(-0.5 * ln(var + eps))
            nc.scalar.activation(out=st_rstd, in_=st_var, func=AF.Ln, bias=eps_sb[:])
            nc.scalar.activation(out=st_rstd, in_=st_rstd, func=AF.Exp, scale=-0.5)
            tap("st_mu", st_mu[:])
            tap("st_rstd", st_rstd[:])
            nc.gpsimd.partition_broadcast(muB[:], st_mu[:])
            nc.gpsimd.partition_broadcast(rstdB[:], st_rstd[:])
            pstat_cm.__exit__(None, None, None)

            # ---- modulate: xmT = ((xT - mu) * rstd) * (1+sc_msa) + sh_msa ----
            xmT = big.tile([128, KT, N], f32, tag="B")
            with tc.tile_pool(name="ptmp", bufs=2) as ptmp:
                for k in range(KT):
                    tmp = ptmp.tile([128, N], f32, tag="tmp")
                    nc.vector.tensor_sub(out=tmp, in0=xT[:, k, :], in1=muB[:])
                    nc.vector.tensor_mul(out=tmp, in0=tmp, in1=rstdB[:])
                    nc.scalar.activation(out=xmT[:, k, :].bitcast(f32r), in_=tmp,
                                         func=AF.Identity,
                                         scale=sc1p[:, k:k + 1],
                                         bias=cmT[:, k:k + 1])

            tap("xmT0", xmT[:, 0, :])
            pmub_cm.__exit__(None, None, None)

            # ---- qT / kT ----
            qT = big.tile([128, KT, TOK], f32, tag="D")
            kTt = big.tile([128, KT, N], f32, tag="A")
            with tc.tile_pool(name="pwqk", bufs=3) as pwqk:
                for mt in range(MT_QK):
                    wt = pwqk.tile([128, KT, 128], f32, tag="wqk")
                    nc.sync.dma_start(out=wt[:].bitcast(f32r), in_=dwqk.ap()[mt].bitcast(f32r))
                    is_q = mt < KT
                    for nch in ([0] if is_q else [0, 1]):
                        ps = pmm.tile([128, 512], f32, tag="mm")
                        for k in range(KT):
                            nc.tensor.matmul(
                                ps, r(wt[:, k, :]),
                                r(xmT[:, k, nch * 512:(nch + 1) * 512]),
                                start=(k == 0), stop=(k == KT - 1))
                        if is_q:
                            dst = qT[:, mt, :]
                        else:
                            dst = kTt[:, mt - KT, nch * 512:(nch + 1) * 512]
                        nc.vector.tensor_scalar(
                            out=dst.bitcast(f32r), in0=ps[:],
                            scalar1=bqkT[:, mt:mt + 1],
                            scalar2=None, op0=OP.add)

            tap("qT0", qT[:, 0, :])
            tap("kT0", kTt[:, 0, 0:512])

            # ---- v (natural layout, augmented with ones column per head) ----
            v_aug = big.tile([128, KT, VA], f32, tag="C")
            nc.vector.memset(
                v_aug[:].rearrange("p k (h c) -> p k h c", c=D + 1)[:, :, :, D:D + 1]
                .bitcast(mybir.dt.uint32), 0x3F800000)
            with tc.tile_pool(name="pwv", bufs=1) as pwv:
                for nch in range(2):
                    # half-slabs of w_v for this output-column chunk
                    wv_slabs = []
                    for k in range(KT):
                        wv = pwv.tile([128, 512], f32, tag=f"wv{k}")
                        nc.sync.dma_start(
                            out=wv[:].bitcast(f32r),
                            in_=dwv.ap()[k * 128:(k + 1) * 128,
                                         nch * 512:(nch + 1) * 512].bitcast(f32r))
                        wv_slabs.append(wv)
                    for mtok in range(KT):
                        ps = pmm.tile([128, 512], f32, tag="mm")
                        for k in range(KT):
                            nc.tensor.matmul(
                                ps, r(xmT[:, k, mtok * 128:(mtok + 1) * 128]),
                                r(wv_slabs[k][:]),
                                start=(k == 0), stop=(k == KT - 1))
                        for h8 in range(8):
                            hh = nch * 8 + h8
                            nc.any.tensor_copy(
                                out=v_aug[:, mtok, hh * (D + 1):hh * (D + 1) + D]
                                .bitcast(f32r),
                                in_=ps[:, h8 * D:(h8 + 1) * D])

            tap("vaug0", v_aug[:, 0, :])

            # ---- attention: per head pair ----
            att = big.tile([128, KT, TOK], f32, tag="E")
            with tc.tile_pool(name="pexp", bufs=3) as pexp, \
                 tc.tile_pool(name="pnorm", bufs=2) as pnorm:
                for t in range(KT):
                    psA = pav.tile([D + 1, 512], f32, tag="av")
                    psB = pav.tile([D + 1, 512], f32, tag="av")
                    for kb in range(KT):
                        pss = psc.tile([128, 1024], f32, tag="sc")
                        nc.tensor.matmul(
                            pss[:, 0:512],
                            r(kTt[0:64, t, kb * 128:(kb + 1) * 128]),
                            r(qT[0:64, t, :]), start=True, stop=True)
                        nc.tensor.matmul(
                            pss[:, 512:1024],
                            r(kTt[64:128, t, kb * 128:(kb + 1) * 128]),
                            r(qT[64:128, t, :]), start=True, stop=True)
                        ex = pexp.tile([128, 1024], f32, tag="exp")
                        nc.scalar.activation(out=ex[:].bitcast(f32r), in_=pss[:],
                                             func=AF.Exp, scale=1.0 / np.sqrt(D))
                        hA, hB = 2 * t, 2 * t + 1
                        nc.tensor.matmul(
                            psA, r(v_aug[:, kb, hA * (D + 1):(hA + 1) * (D + 1)]),
                            r(ex[:, 0:512]),
                            start=(kb == 0), stop=(kb == KT - 1))
                        nc.tensor.matmul(
                            psB, r(v_aug[:, kb, hB * (D + 1):(hB + 1) * (D + 1)]),
                            r(ex[:, 512:1024]),
                            start=(kb == 0), stop=(kb == KT - 1))
                    # normalize: recip of row-64 denominators via exp(-ln(d))
                    # (partition_broadcast only writes from out-base 0, so use
                    # two base-0 tiles; DVE handles the out-partition shift)
                    rbs = []
                    for psX in (psA, psB):
                        rx = pnorm.tile([1, TOK], f32, tag="rx")
                        nc.scalar.activation(out=rx, in_=psX[D:D + 1, :], func=AF.Ln)
                        nc.scalar.activation(out=rx, in_=rx, func=AF.Exp, scale=-1.0)
                        rb = pnorm.tile([64, TOK], f32, tag="rb")
                        nc.gpsimd.partition_broadcast(rb[:], rx[:])
                        rbs.append(rb)
                    nc.vector.tensor_mul(out=att[0:64, t, :].bitcast(f32r),
                                         in0=psA[0:D, :], in1=rbs[0][:])
                    nc.vector.tensor_mul(out=att[64:128, t, :].bitcast(f32r),
                                         in0=psB[0:D, :], in1=rbs[1][:])
                    nc.scalar.activation(out=att[:, t, :].bitcast(f32r),
                                         in_=att[:, t, :],
                                         func=AF.Identity, bias=bvT[:, t:t + 1])

            tap("att0", att[:, 0, :])
            tap("att7", att[:, 7, :])

            # ---- proj -> yT = (proj + b_proj) * (1 + gate_msa) ----
            yT = big.tile([128, KT, TOK], f32, tag="F")
            with tc.tile_pool(name="pwproj", bufs=2) as pwproj:
                for mt in range(KT):
                    wt = pwproj.tile([128, KT, 128], f32, tag="wproj")
                    nc.sync.dma_start(out=wt[:].bitcast(f32r), in_=dwproj.ap()[mt].bitcast(f32r))
                    ps = pmm.tile([128, 512], f32, tag="mm")
                    for k in range(KT):
                        nc.tensor.matmul(ps, r(wt[:, k, :]), r(att[:, k, :]),
                                         start=(k == 0), stop=(k == KT - 1))
                    nc.vector.tensor_scalar(
                        out=yT[:, mt, :].bitcast(f32r), in0=ps[:],
                        scalar1=bprojT[:, mt:mt + 1], scalar2=g1p[:, mt:mt + 1],
                        op0=OP.add, op1=OP.mult)

            # ---- LN2 stats + modulate -> xm2T ----
            pmub2_cm = tc.tile_pool(name="pmub2", bufs=1)
            pmub2 = pmub2_cm.__enter__()
            muB2 = pmub2.tile([128, TOK], f32)
            rstdB2 = pmub2.tile([128, TOK], f32)
            pstat2_cm = tc.tile_pool(name="pstat2", bufs=1)
            pstat2 = pstat2_cm.__enter__()
            st2_mu = pstat2.tile([1, TOK], f32)
            st2_e2 = pstat2.tile([1, TOK], f32)
            st2_var = pstat2.tile([1, TOK], f32)
            st2_rstd = pstat2.tile([1, TOK], f32)
            with tc.tile_pool(name="pysq", bufs=2) as pysq:
                ps = pmm.tile([1, 512], f32, tag="mm")
                for k in range(KT):
                    nc.tensor.matmul(ps, r(ones[:]), r(yT[:, k, :]),
                                     start=(k == 0), stop=(k == KT - 1))
                nc.scalar.activation(out=st2_mu, in_=ps[:], func=AF.Copy,
                                     scale=1.0 / H)
                ps = pmm.tile([1, 512], f32, tag="mm")
                for k in range(KT):
                    ysq = pysq.tile([128, TOK], bf16, tag="ysq")
                    nc.vector.tensor_mul(out=ysq, in0=yT[:, k, :], in1=yT[:, k, :])
                    nc.tensor.matmul(ps, ones_bf[:], ysq[:],
                                     start=(k == 0), stop=(k == KT - 1))
                nc.scalar.activation(out=st2_e2, in_=ps[:], func=AF.Copy,
                                     scale=1.0 / H)
            nc.vector.tensor_mul(out=st2_var, in0=st2_mu, in1=st2_mu)
            nc.vector.tensor_sub(out=st2_var, in0=st2_e2, in1=st2_var)
            nc.scalar.activation(out=st2_rstd, in_=st2_var, func=AF.Ln, bias=eps_sb[:])
            nc.scalar.activation(out=st2_rstd, in_=st2_rstd, func=AF.Exp, scale=-0.5)
            nc.gpsimd.partition_broadcast(muB2[:], st2_mu[:])
            nc.gpsimd.partition_broadcast(rstdB2[:], st2_rstd[:])
            pstat2_cm.__exit__(None, None, None)

            xm2T = big.tile([128, KT, TOK], f32, tag="D")
            with tc.tile_pool(name="ptmp2", bufs=2) as ptmp2:
                for k in range(KT):
                    tmp = ptmp2.tile([128, TOK], f32, tag="tmp2")
                    nc.vector.tensor_sub(out=tmp, in0=yT[:, k, :], in1=muB2[:])
                    nc.vector.tensor_mul(out=tmp, in0=tmp, in1=rstdB2[:])
                    nc.scalar.activation(out=xm2T[:, k, :].bitcast(f32r), in_=tmp,
                                         func=AF.Identity,
                                         scale=sc2p[:, k:k + 1],
                                         bias=cmT[:, 24 + k:24 + k + 1])

            tap("yT0", yT[:, 0, :])
            tap("xm2T0", xm2T[:, 0, :])
            pmub2_cm.__exit__(None, None, None)

            # ---- mlp1: hidT = gelu(w1T @ xm2T + b1) (bf16) ----
            hidT = big.tile([128, MT_FF, TOK], bf16, tag="B")
            with tc.tile_pool(name="pw1", bufs=3) as pw1:
                for mt in range(MT_FF):
                    wt = pw1.tile([128, KT, 128], f32, tag="w1")
                    nc.sync.dma_start(out=wt[:].bitcast(f32r), in_=dw1.ap()[mt].bitcast(f32r))
                    ps = pmm.tile([128, 512], f32, tag="mm")
                    for k in range(KT):
                        nc.tensor.matmul(ps, r(wt[:, k, :]), r(xm2T[:, k, :]),
                                         start=(k == 0), stop=(k == KT - 1))
                    nc.scalar.activation(out=hidT[:, mt, :], in_=ps[:],
                                         func=AF.Gelu, bias=b1T[:, mt:mt + 1])

            tap("hid0", hidT[:, 0, :])

            # ---- mlp2 + residual -> out ----
            with tc.tile_pool(name="pw2", bufs=2) as pw2, \
                 tc.tile_pool(name="pout", bufs=3) as pout:
                for mt in range(KT):
                    wt = pw2.tile([128, MT_FF, 128], bf16, tag="w2")
                    nc.sync.dma_start(out=wt, in_=dw2.ap()[mt])
                    ps = pmm.tile([128, 512], f32, tag="mm")
                    for kk in range(MT_FF):
                        nc.tensor.matmul(ps, wt[:, kk, :], hidT[:, kk, :],
                                         start=(kk == 0), stop=(kk == MT_FF - 1))
                    ot = pout.tile([128, TOK], f32, tag="out")
                    nc.vector.tensor_scalar(
                        out=ot, in0=ps[:], scalar1=b2T[:, mt:mt + 1],
                        scalar2=cmT[:, 40 + mt:40 + mt + 1],
                        op0=OP.add, op1=OP.mult)
                    nc.vector.tensor_add(out=ot, in0=ot, in1=yT[:, mt, :])
                    nc.sync.dma_start(out=doutT.ap()[mt * 128:(mt + 1) * 128, :],
                                      in_=ot)

    nc.compile()
    return nc


def _prep_shared(w_mod, b_mod, w_qkv, b_qkv, w_proj, b_proj, w1, b1, w2, b2):
    c32 = np.ascontiguousarray
    f32 = np.float32
    shared = {
        "wmod": c32(w_mod.reshape(8, 128, 12, 512).transpose(2, 0, 1, 3)).astype(f32, copy=False),
        "bmodT": c32(b_mod.reshape(48, 128).T).astype(f32, copy=False),
        "wqk": c32(w_qkv[:, :2048].reshape(8, 128, 16, 128).transpose(2, 1, 0, 3)).astype(f32, copy=False),
        "bqkT": c32(b_qkv[:2048].reshape(16, 128).T).astype(f32, copy=False),
        "wv": c32(w_qkv[:, 2048:]).astype(f32, copy=False),
        "bvT": c32(b_qkv[2048:].reshape(8, 128).T).astype(f32, copy=False),
        "wproj": c32(w_proj.reshape(8, 128, 8, 128).transpose(2, 1, 0, 3)).astype(f32, copy=False),
        "bprojT": c32(b_proj.reshape(8, 128).T).astype(f32, copy=False),
        "w1": c32(w1.reshape(8, 128, 32, 128).transpose(2, 1, 0, 3)).astype(f32, copy=False),
        "b1T": c32(b1.reshape(32, 128).T).astype(f32, copy=False),
        "w2": c32(w2.reshape(32, 128, 8, 128).transpose(2, 1, 0, 3)).astype(ml_dtypes.bfloat16),
        "b2T": c32(b2.reshape(8, 128).T).astype(f32, copy=False),
    }
    return shared


def make_in_maps(x, c, w_mod, b_mod, w_qkv, b_qkv, w_proj, b_proj, w1, b1, w2, b2):
    x = np.asarray(x, np.float32)
    c = np.asarray(c, np.float32)
    shared = _prep_shared(np.asarray(w_mod, np.float32), np.asarray(b_mod, np.float32),
                          np.asarray(w_qkv, np.float32), np.asarray(b_qkv, np.float32),
                          np.asarray(w_proj, np.float32), np.asarray(b_proj, np.float32),
                          np.asarray(w1, np.float32), np.asarray(b1, np.float32),
                          np.asarray(w2, np.float32), np.asarray(b2, np.float32))
    in_maps = []
    for ci in range(NCORES):
        b, h = divmod(ci, 2)
        xTb = x[b].T
        xcore = np.ascontiguousarray(
            np.concatenate([xTb[:, h * TOK:(h + 1) * TOK],
                            xTb[:, (1 - h) * TOK:(2 - h) * TOK]], axis=1))
        cTb = np.ascontiguousarray(c[b].reshape(8, 128).T)
        m = dict(shared)
        m["xT"] = xcore
        m["cT"] = cTb
        in_maps.append(m)
    return in_maps


def gather(results):
    out = np.empty((B, N, H), np.float32)
    for ci in range(NCORES):
        b, h = divmod(ci, 2)
        out[b, h * TOK:(h + 1) * TOK, :] = results[ci]["outT"].T
    return out


def get_nc(taps=()):
    key = ("nc", tuple(sorted(taps)))
    if key not in _CACHE:
        _CACHE[key] = _build(taps=taps)
    return _CACHE[key]


def kernel(**inputs):
    from concourse import bass_utils
    nc = get_nc()
    in_maps = make_in_maps(**inputs)
    res = bass_utils.run_bass_kernel_spmd(nc, in_maps,
                                          core_ids=list(range(NCORES)))
    return gather(res.results)


# revision 15
# speedup vs baseline: 1.1793x; 1.1793x over previous
"""DiT block kernel for Trainium2, 8 NeuronCores.

Sharding: core ci = (batch b = ci//2, token-half h = ci%2). Each core computes
the full DiT block for 512 "q" tokens of one batch. Attention needs all 1024
k/v tokens of the batch, so the per-batch LN/qkv-kv work is duplicated across
the 2 cores sharing a batch (≈14% FLOP overhead, zero collectives).

Everything on-device runs in a "transposed" activation layout [feature, token]
so that per-feature modulation vectors (adaLN shift/scale/gate) and all biases
are per-partition scalars, and all matmuls consume weights in their natural
or host-pre-tiled layout with zero on-device transposes of big tensors.

Key layout identities (out = lhsT.T @ rhs on the PE):
  cm       [1,6H]    : lhsT = silu(c).T k-slabs,  rhs = w_mod slabs
  q/kT     [C, tok]  : lhsT = w_qkv tiles,        rhs = xmT
  v        [tok, C]  : lhsT = xmT,                rhs = w_v slabs
  scoresT  [kt, qt]  : lhsT = kT head slice,      rhs = qT head slice (K=64)
  att_avT  [65, qt]  : lhsT = [v_head | ones],    rhs = exp(scoresT)  (row 64
                       accumulates the softmax denominator for free)
  yT       [H, tok]  : lhsT = w_proj tiles,       rhs = att_outT
  hidT     [dff,tok] : lhsT = w1 tiles,           rhs = xm2T
  finalT   [H, tok]  : lhsT = w2 tiles (bf16),    rhs = hidT (bf16)

LN stats (mean / mean-of-squares over the H partition dim) are computed with
ones-vector matmuls on the PE; rstd = exp(-0.5*ln(var+eps)) on ACT (keeps the
Ln/Exp table set hot, avoids the banned Rsqrt).
"""

import sys

for _p in ("/opt/trn_rl_repo",):
    if _p not in sys.path:
        sys.path.insert(0, _p)

import numpy as np
import ml_dtypes

B, N, H = 4, 1024, 1024
NH, D = 16, 64
DFF = 4 * H
EPS = 1e-5
NCORES = 8
TOK = N // 2      # q tokens per core
KT = H // 128     # 8  H-tiles
MT_QK = 16        # q+k column tiles
MT_FF = DFF // 128  # 32
VA = NH * (D + 1)   # 1040 v_aug columns

_CACHE = {}


def _build(taps=()):
    import concourse.bacc as bacc
    import concourse.tile as tile
    import concourse.mybir as mybir
    from concourse.masks import make_identity

    f32 = mybir.dt.float32
    f32r = mybir.dt.float32r
    bf16 = mybir.dt.bfloat16
    AF = mybir.ActivationFunctionType
    OP = mybir.AluOpType

    nc = bacc.Bacc("TRN2", target_bir_lowering=False, debug=False,
                   num_devices=NCORES)

    dxT = nc.dram_tensor("xT", [H, N], f32, kind="ExternalInput")
    dcT = nc.dram_tensor("cT", [128, KT], f32, kind="ExternalInput")
    dwmod = nc.dram_tensor("wmod", [12, KT, 128, 512], bf16, kind="ExternalInput")
    dbmodT = nc.dram_tensor("bmodT", [128, 48], f32, kind="ExternalInput")
    dwqk = nc.dram_tensor("wqk", [MT_QK, 128, KT, 128], bf16, kind="ExternalInput")
    dbqkT = nc.dram_tensor("bqkT", [128, MT_QK], f32, kind="ExternalInput")
    dwv = nc.dram_tensor("wv", [H, H], bf16, kind="ExternalInput")
    dbvT = nc.dram_tensor("bvT", [128, KT], f32, kind="ExternalInput")
    dwproj = nc.dram_tensor("wproj", [KT, 128, KT, 128], bf16, kind="ExternalInput")
    dbprojT = nc.dram_tensor("bprojT", [128, KT], f32, kind="ExternalInput")
    dw1 = nc.dram_tensor("w1", [MT_FF, 128, KT, 128], bf16, kind="ExternalInput")
    db1T = nc.dram_tensor("b1T", [128, MT_FF], f32, kind="ExternalInput")
    dw2 = nc.dram_tensor("w2", [KT, 128, MT_FF, 128], bf16, kind="ExternalInput")
    db2T = nc.dram_tensor("b2T", [128, KT], f32, kind="ExternalInput")
    doutT = nc.dram_tensor("outT", [H, TOK], f32, kind="ExternalOutput")

    _tapt = {}

    def tap(name, ap, shape=None, dtype=None):
        if name not in taps:
            return
        if name not in _tapt:
            _tapt[name] = nc.dram_tensor(
                "tap_" + name, list(shape or ap.shape), dtype or ap.dtype,
                kind="ExternalOutput")
        nc.sync.dma_start(out=_tapt[name].ap(), in_=ap)

    def r(ap):
        return ap.bitcast(f32r)

    with tile.TileContext(nc) as tc:
        from contextlib import ExitStack
        es = ExitStack()
        with es:
            pconst = es.enter_context(tc.tile_pool(name="const", bufs=1))
            big = es.enter_context(tc.tile_pool(name="big", bufs=1))
            pmm = es.enter_context(tc.tile_pool(name="pmm", bufs=2, space="PSUM"))
            psc = es.enter_context(tc.tile_pool(name="psc", bufs=2, space="PSUM"))
            pav = es.enter_context(tc.tile_pool(name="pav", bufs=2, space="PSUM"))

            # ---- constants ----
            ident = pconst.tile([128, 128], f32)
            make_identity(nc, ident)
            ones = pconst.tile([128, 1], f32)
            nc.vector.memset(ones[:].bitcast(mybir.dt.uint32), 0x3F800000)
            ones_bf = pconst.tile([128, 1], bf16)
            nc.vector.memset(ones_bf, 1.0)

            cT_sb = pconst.tile([128, KT], f32)
            nc.sync.dma_start(out=cT_sb, in_=dcT.ap())
            bmodT = pconst.tile([128, 48], f32)
            nc.sync.dma_start(out=bmodT, in_=dbmodT.ap())
            bqkT = pconst.tile([128, MT_QK], f32)
            nc.sync.dma_start(out=bqkT, in_=dbqkT.ap())
            bvT = pconst.tile([128, KT], f32)
            nc.sync.dma_start(out=bvT, in_=dbvT.ap())
            bprojT = pconst.tile([128, KT], f32)
            nc.sync.dma_start(out=bprojT, in_=dbprojT.ap())
            b1T = pconst.tile([128, MT_FF], f32)
            nc.sync.dma_start(out=b1T, in_=db1T.ap())
            b2T = pconst.tile([128, KT], f32)
            nc.sync.dma_start(out=b2T, in_=db2T.ap())

            silucT = pconst.tile([128, KT], bf16)
            nc.scalar.activation(out=silucT, in_=cT_sb, func=AF.Silu)
            eps_sb = pconst.tile([1, 1], f32)
            nc.vector.memset(eps_sb, EPS)

            # ---- xT load ----
            xT = big.tile([128, KT, N], f32, tag="A")
            for k in range(KT):
                nc.sync.dma_start(out=xT[:, k, :].bitcast(f32r),
                                  in_=dxT.ap()[k * 128:(k + 1) * 128, :].bitcast(f32r))

            # ---- cm = silu(c) @ w_mod ; staged via DRAM into [48, 128] ----
            cm_rs = pconst.tile([48, 128], f32)
            with tc.tile_pool(name="pwmod", bufs=8) as pwmod, \
                 tc.tile_pool(name="pcmb", bufs=2) as pcmb, \
                 tc.tile_pool(name="pcmd", bufs=1, space="DRAM") as pcmd:
                cmdram = pcmd.tile([12, 512], f32)
                for nchk in range(12):
                    ps = pmm.tile([1, 512], f32, tag="mm")
                    for k in range(KT):
                        wt = pwmod.tile([128, 512], bf16, tag="wmod")
                        nc.sync.dma_start(out=wt, in_=dwmod.ap()[nchk, k])
                        nc.tensor.matmul(ps, silucT[:, k:k + 1], wt[:],
                                         start=(k == 0), stop=(k == KT - 1))
                    cb = pcmb.tile([1, 512], f32, tag="cmb")
                    nc.any.tensor_copy(out=cb, in_=ps[:])
                    nc.sync.dma_start(out=cmdram[nchk:nchk + 1, :], in_=cb[:])
                nc.sync.dma_start(
                    out=cm_rs,
                    in_=cmdram[:].rearrange("a (b f) -> (a b) f", b=4))
            tap("cm_rs", cm_rs[:])
            ps_cmT = pmm.tile([128, 48], f32, tag="mm")
            nc.tensor.transpose(ps_cmT, cm_rs[:], ident[0:48, 0:48])
            cmT = pconst.tile([128, 48], f32)
            nc.vector.tensor_add(out=cmT, in0=ps_cmT[:], in1=bmodT[:])
            tap("cmT_raw", cmT[:])
            # cmT columns: v*8+t ; v: 0 sh_msa, 1 sc_msa, 2 g_msa, 3 sh_mlp,
            # 4 sc_mlp, 5 g_mlp
            sc1p = pconst.tile([128, KT], f32)
            nc.vector.tensor_scalar_add(out=sc1p, in0=cmT[:, 8:16], scalar1=1.0)
            g1p = pconst.tile([128, KT], f32)
            nc.vector.tensor_scalar_add(out=g1p, in0=cmT[:, 16:24], scalar1=1.0)
            sc2p = pconst.tile([128, KT], f32)
            nc.vector.tensor_scalar_add(out=sc2p, in0=cmT[:, 32:40], scalar1=1.0)
            tap("cmT", cmT[:])

            # ---- LN1 stats over H (partition dim) via ones-matmuls ----
            pmub_cm = tc.tile_pool(name="pmub", bufs=1)
            pmub = pmub_cm.__enter__()
            muB = pmub.tile([128, N], f32)
            rstdB = pmub.tile([128, N], f32)
            pstat_cm = tc.tile_pool(name="pstat", bufs=1)
            pstat = pstat_cm.__enter__()
            st_mu = pstat.tile([1, N], f32)
            st_e2 = pstat.tile([1, N], f32)
            st_var = pstat.tile([1, N], f32)
            st_rstd = pstat.tile([1, N], f32)

            with tc.tile_pool(name="pxsq", bufs=2) as pxsq:
                for nch in range(2):
                    ps = pmm.tile([1, 512], f32, tag="mm")
                    for k in range(KT):
                        nc.tensor.matmul(
                            ps, r(ones[:]), r(xT[:, k, nch * 512:(nch + 1) * 512]),
                            start=(k == 0), stop=(k == KT - 1))
                    nc.scalar.activation(out=st_mu[0:1, nch * 512:(nch + 1) * 512],
                                         in_=ps[:], func=AF.Copy, scale=1.0 / H)
                for nch in range(2):
                    ps = pmm.tile([1, 512], f32, tag="mm")
                    for k in range(KT):
                        xsq = pxsq.tile([128, 512], bf16, tag="xsq")
                        nc.vector.tensor_mul(
                            out=xsq, in0=xT[:, k, nch * 512:(nch + 1) * 512],
                            in1=xT[:, k, nch * 512:(nch + 1) * 512])
                        nc.tensor.matmul(ps, ones_bf[:], xsq[:],
                                         start=(k == 0), stop=(k == KT - 1))
                    nc.scalar.activation(out=st_e2[0:1, nch * 512:(nch + 1) * 512],
                                         in_=ps[:], func=AF.Copy, scale=1.0 / H)
            nc.vector.tensor_mul(out=st_var, in0=st_mu, in1=st_mu)
            nc.vector.tensor_sub(out=st_var, in0=st_e2, in1=st_var)
            # rstd = exp(-0.5 * ln(var + eps))
            nc.scalar.activation(out=st_rstd, in_=st_var, func=AF.Ln, bias=eps_sb[:])
            nc.scalar.activation(out=st_rstd, in_=st_rstd, func=AF.Exp, scale=-0.5)
            tap("st_mu", st_mu[:])
            tap("st_rstd", st_rstd[:])
            nc.gpsimd.partition_broadcast(muB[:], st_mu[:])
            nc.gpsimd.partition_broadcast(rstdB[:], st_rstd[:])
            pstat_cm.__exit__(None, None, None)

            # ---- modulate: xmT = ((xT - mu) * rstd) * (1+sc_msa) + sh_msa ----
            xmT = big.tile([128, KT, N], bf16, tag="B")
            with tc.tile_pool(name="ptmp", bufs=2) as ptmp:
                for k in range(KT):
                    tmp = ptmp.tile([128, N], f32, tag="tmp")
                    nc.vector.tensor_sub(out=tmp, in0=xT[:, k, :], in1=muB[:])
                    nc.vector.tensor_mul(out=tmp, in0=tmp, in1=rstdB[:])
                    nc.scalar.activation(out=xmT[:, k, :], in_=tmp,
                                         func=AF.Identity,
                                         scale=sc1p[:, k:k + 1],
                                         bias=cmT[:, k:k + 1])

            tap("xmT0", xmT[:, 0, :])
            pmub_cm.__exit__(None, None, None)

            # ---- qT / kT ----
            qT = big.tile([128, KT, TOK], bf16, tag="D")
            kTt = big.tile([128, KT, N], bf16, tag="A")
            with tc.tile_pool(name="pwqk", bufs=3) as pwqk:
                for mt in range(MT_QK):
                    wt = pwqk.tile([128, KT, 128], bf16, tag="wqk")
                    nc.sync.dma_start(out=wt, in_=dwqk.ap()[mt])
                    is_q = mt < KT
                    for nch in ([0] if is_q else [0, 1]):
                        ps = pmm.tile([128, 512], f32, tag="mm")
                        for k in range(KT):
                            nc.tensor.matmul(
                                ps, wt[:, k, :],
                                xmT[:, k, nch * 512:(nch + 1) * 512],
                                start=(k == 0), stop=(k == KT - 1))
                        if is_q:
                            dst = qT[:, mt, :]
                        else:
                            dst = kTt[:, mt - KT, nch * 512:(nch + 1) * 512]
                        nc.vector.tensor_scalar(
                            out=dst, in0=ps[:],
                            scalar1=bqkT[:, mt:mt + 1],
                            scalar2=None, op0=OP.add)

            tap("qT0", qT[:, 0, :])
            tap("kT0", kTt[:, 0, 0:512])

            # ---- v (natural layout, augmented with ones column per head) ----
            v_aug = big.tile([128, KT, VA], bf16, tag="C")
            nc.vector.memset(
                v_aug[:].rearrange("p k (h c) -> p k h c", c=D + 1)[:, :, :, D:D + 1],
                1.0)
            with tc.tile_pool(name="pwv", bufs=1) as pwv:
                for nch in range(2):
                    # half-slabs of w_v for this output-column chunk
                    wv_slabs = []
                    for k in range(KT):
                        wv = pwv.tile([128, 512], bf16, tag=f"wv{k}")
                        nc.sync.dma_start(
                            out=wv,
                            in_=dwv.ap()[k * 128:(k + 1) * 128,
                                         nch * 512:(nch + 1) * 512])
                        wv_slabs.append(wv)
                    for mtok in range(KT):
                        ps = pmm.tile([128, 512], f32, tag="mm")
                        for k in range(KT):
                            nc.tensor.matmul(
                                ps, xmT[:, k, mtok * 128:(mtok + 1) * 128],
                                wv_slabs[k][:],
                                start=(k == 0), stop=(k == KT - 1))
                        for h8 in range(8):
                            hh = nch * 8 + h8
                            nc.any.tensor_copy(
                                out=v_aug[:, mtok, hh * (D + 1):hh * (D + 1) + D],
                                in_=ps[:, h8 * D:(h8 + 1) * D])

            tap("vaug0", v_aug[:, 0, :])

            # ---- attention: per head pair ----
            att = big.tile([128, KT, TOK], bf16, tag="E")
            with tc.tile_pool(name="pexp", bufs=3) as pexp, \
                 tc.tile_pool(name="pnorm", bufs=2) as pnorm:
                for t in range(KT):
                    psA = pav.tile([D + 1, 512], f32, tag="av")
                    psB = pav.tile([D + 1, 512], f32, tag="av")
                    for kb in range(KT):
                        pss = psc.tile([128, 1024], f32, tag="sc")
                        nc.tensor.matmul(
                            pss[:, 0:512],
                            kTt[0:64, t, kb * 128:(kb + 1) * 128],
                            qT[0:64, t, :], start=True, stop=True)
                        nc.tensor.matmul(
                            pss[:, 512:1024],
                            kTt[64:128, t, kb * 128:(kb + 1) * 128],
                            qT[64:128, t, :], start=True, stop=True)
                        ex = pexp.tile([128, 1024], bf16, tag="exp")
                        nc.scalar.activation(out=ex, in_=pss[:],
                                             func=AF.Exp, scale=1.0 / np.sqrt(D))
                        hA, hB = 2 * t, 2 * t + 1
                        nc.tensor.matmul(
                            psA, v_aug[:, kb, hA * (D + 1):(hA + 1) * (D + 1)],
                            ex[:, 0:512],
                            start=(kb == 0), stop=(kb == KT - 1))
                        nc.tensor.matmul(
                            psB, v_aug[:, kb, hB * (D + 1):(hB + 1) * (D + 1)],
                            ex[:, 512:1024],
                            start=(kb == 0), stop=(kb == KT - 1))
                    # normalize: recip of row-64 denominators via exp(-ln(d))
                    # (partition_broadcast only writes from out-base 0, so use
                    # two base-0 tiles; DVE handles the out-partition shift)
                    rbs = []
                    for psX in (psA, psB):
                        rx = pnorm.tile([1, TOK], f32, tag="rx")
                        nc.scalar.activation(out=rx, in_=psX[D:D + 1, :], func=AF.Ln)
                        nc.scalar.activation(out=rx, in_=rx, func=AF.Exp, scale=-1.0)
                        rb = pnorm.tile([64, TOK], f32, tag="rb")
                        nc.gpsimd.partition_broadcast(rb[:], rx[:])
                        rbs.append(rb)
                    nc.vector.tensor_mul(out=att[0:64, t, :],
                                         in0=psA[0:D, :], in1=rbs[0][:])
                    nc.vector.tensor_mul(out=att[64:128, t, :],
                                         in0=psB[0:D, :], in1=rbs[1][:])
                    nc.scalar.activation(out=att[:, t, :],
                                         in_=att[:, t, :],
                                         func=AF.Identity, bias=bvT[:, t:t + 1])

            tap("att0", att[:, 0, :])
            tap("att7", att[:, 7, :])

            # ---- proj -> yT = (proj + b_proj) * (1 + gate_msa) ----
            yT = big.tile([128, KT, TOK], f32, tag="F")
            with tc.tile_pool(name="pwproj", bufs=2) as pwproj:
                for mt in range(KT):
                    wt = pwproj.tile([128, KT, 128], bf16, tag="wproj")
                    nc.sync.dma_start(out=wt, in_=dwproj.ap()[mt])
                    ps = pmm.tile([128, 512], f32, tag="mm")
                    for k in range(KT):
                        nc.tensor.matmul(ps, wt[:, k, :], att[:, k, :],
                                         start=(k == 0), stop=(k == KT - 1))
                    nc.vector.tensor_scalar(
                        out=yT[:, mt, :].bitcast(f32r), in0=ps[:],
                        scalar1=bprojT[:, mt:mt + 1], scalar2=g1p[:, mt:mt + 1],
                        op0=OP.add, op1=OP.mult)  # f32r-rounded for LN2 mean mm

            # ---- LN2 stats + modulate -> xm2T ----
            pmub2_cm = tc.tile_pool(name="pmub2", bufs=1)
            pmub2 = pmub2_cm.__enter__()
            muB2 = pmub2.tile([128, TOK], f32)
            rstdB2 = pmub2.tile([128, TOK], f32)
            pstat2_cm = tc.tile_pool(name="pstat2", bufs=1)
            pstat2 = pstat2_cm.__enter__()
            st2_mu = pstat2.tile([1, TOK], f32)
            st2_e2 = pstat2.tile([1, TOK], f32)
            st2_var = pstat2.tile([1, TOK], f32)
            st2_rstd = pstat2.tile([1, TOK], f32)
            with tc.tile_pool(name="pysq", bufs=2) as pysq:
                ps = pmm.tile([1, 512], f32, tag="mm")
                for k in range(KT):
                    nc.tensor.matmul(ps, r(ones[:]), r(yT[:, k, :]),
                                     start=(k == 0), stop=(k == KT - 1))
                nc.scalar.activation(out=st2_mu, in_=ps[:], func=AF.Copy,
                                     scale=1.0 / H)
                ps = pmm.tile([1, 512], f32, tag="mm")
                for k in range(KT):
                    ysq = pysq.tile([128, TOK], bf16, tag="ysq")
                    nc.vector.tensor_mul(out=ysq, in0=yT[:, k, :], in1=yT[:, k, :])
                    nc.tensor.matmul(ps, ones_bf[:], ysq[:],
                                     start=(k == 0), stop=(k == KT - 1))
                nc.scalar.activation(out=st2_e2, in_=ps[:], func=AF.Copy,
                                     scale=1.0 / H)
            nc.vector.tensor_mul(out=st2_var, in0=st2_mu, in1=st2_mu)
            nc.vector.tensor_sub(out=st2_var, in0=st2_e2, in1=st2_var)
            nc.scalar.activation(out=st2_rstd, in_=st2_var, func=AF.Ln, bias=eps_sb[:])
            nc.scalar.activation(out=st2_rstd, in_=st2_rstd, func=AF.Exp, scale=-0.5)
            nc.gpsimd.partition_broadcast(muB2[:], st2_mu[:])
            nc.gpsimd.partition_broadcast(rstdB2[:], st2_rstd[:])
            pstat2_cm.__exit__(None, None, None)

            xm2T = big.tile([128, KT, TOK], bf16, tag="D")
            with tc.tile_pool(name="ptmp2", bufs=2) as ptmp2:
                for k in range(KT):
                    tmp = ptmp2.tile([128, TOK], f32, tag="tmp2")
                    nc.vector.tensor_sub(out=tmp, in0=yT[:, k, :], in1=muB2[:])
                    nc.vector.tensor_mul(out=tmp, in0=tmp, in1=rstdB2[:])
                    nc.scalar.activation(out=xm2T[:, k, :], in_=tmp,
                                         func=AF.Identity,
                                         scale=sc2p[:, k:k + 1],
                                         bias=cmT[:, 24 + k:24 + k + 1])

            tap("yT0", yT[:, 0, :])
            tap("xm2T0", xm2T[:, 0, :])
            pmub2_cm.__exit__(None, None, None)

            # ---- mlp1: hidT = gelu(w1T @ xm2T + b1) (bf16) ----
            hidT = big.tile([128, MT_FF, TOK], bf16, tag="B")
            with tc.tile_pool(name="pw1", bufs=3) as pw1:
                for mt in range(MT_FF):
                    wt = pw1.tile([128, KT, 128], bf16, tag="w1")
                    nc.sync.dma_start(out=wt, in_=dw1.ap()[mt])
                    ps = pmm.tile([128, 512], f32, tag="mm")
                    for k in range(KT):
                        nc.tensor.matmul(ps, wt[:, k, :], xm2T[:, k, :],
                                         start=(k == 0), stop=(k == KT - 1))
                    nc.scalar.activation(out=hidT[:, mt, :], in_=ps[:],
                                         func=AF.Gelu, bias=b1T[:, mt:mt + 1])

            tap("hid0", hidT[:, 0, :])

            # ---- mlp2 + residual -> out ----
            with tc.tile_pool(name="pw2", bufs=2) as pw2, \
                 tc.tile_pool(name="pout", bufs=3) as pout:
                for mt in range(KT):
                    wt = pw2.tile([128, MT_FF, 128], bf16, tag="w2")
                    nc.sync.dma_start(out=wt, in_=dw2.ap()[mt])
                    ps = pmm.tile([128, 512], f32, tag="mm")
                    for kk in range(MT_FF):
                        nc.tensor.matmul(ps, wt[:, kk, :], hidT[:, kk, :],
                                         start=(kk == 0), stop=(kk == MT_FF - 1))
                    ot = pout.tile([128, TOK], f32, tag="out")
                    nc.vector.tensor_scalar(
                        out=ot, in0=ps[:], scalar1=b2T[:, mt:mt + 1],
                        scalar2=cmT[:, 40 + mt:40 + mt + 1],
                        op0=OP.add, op1=OP.mult)
                    nc.vector.tensor_add(out=ot, in0=ot, in1=yT[:, mt, :])
                    nc.sync.dma_start(out=doutT.ap()[mt * 128:(mt + 1) * 128, :],
                                      in_=ot)

    nc.compile()
    return nc


def _prep_shared(w_mod, b_mod, w_qkv, b_qkv, w_proj, b_proj, w1, b1, w2, b2):
    c32 = np.ascontiguousarray
    f32 = np.float32
    shared = {
        "wmod": c32(w_mod.reshape(8, 128, 12, 512).transpose(2, 0, 1, 3)).astype(ml_dtypes.bfloat16),
        "bmodT": c32(b_mod.reshape(48, 128).T).astype(f32, copy=False),
        "wqk": c32(w_qkv[:, :2048].reshape(8, 128, 16, 128).transpose(2, 1, 0, 3)).astype(ml_dtypes.bfloat16),
        "bqkT": c32(b_qkv[:2048].reshape(16, 128).T).astype(f32, copy=False),
        "wv": c32(w_qkv[:, 2048:]).astype(ml_dtypes.bfloat16),
        "bvT": c32(b_qkv[2048:].reshape(8, 128).T).astype(f32, copy=False),
        "wproj": c32(w_proj.reshape(8, 128, 8, 128).transpose(2, 1, 0, 3)).astype(ml_dtypes.bfloat16),
        "bprojT": c32(b_proj.reshape(8, 128).T).astype(f32, copy=False),
        "w1": c32(w1.reshape(8, 128, 32, 128).transpose(2, 1, 0, 3)).astype(ml_dtypes.bfloat16),
        "b1T": c32(b1.reshape(32, 128).T).astype(f32, copy=False),
        "w2": c32(w2.reshape(32, 128, 8, 128).transpose(2, 1, 0, 3)).astype(ml_dtypes.bfloat16),
        "b2T": c32(b2.reshape(8, 128).T).astype(f32, copy=False),
    }
    return shared


def make_in_maps(x, c, w_mod, b_mod, w_qkv, b_qkv, w_proj, b_proj, w1, b1, w2, b2):
    x = np.asarray(x, np.float32)
    c = np.asarray(c, np.float32)
    shared = _prep_shared(np.asarray(w_mod, np.float32), np.asarray(b_mod, np.float32),
                          np.asarray(w_qkv, np.float32), np.asarray(b_qkv, np.float32),
                          np.asarray(w_proj, np.float32), np.asarray(b_proj, np.float32),
                          np.asarray(w1, np.float32), np.asarray(b1, np.float32),
                          np.asarray(w2, np.float32), np.asarray(b2, np.float32))
    in_maps = []
    for ci in range(NCORES):
        b, h = divmod(ci, 2)
        xTb = x[b].T
        xcore = np.ascontiguousarray(
            np.concatenate([xTb[:, h * TOK:(h + 1) * TOK],
                            xTb[:, (1 - h) * TOK:(2 - h) * TOK]], axis=1))
        cTb = np.ascontiguousarray(c[b].reshape(8, 128).T)
        m = dict(shared)
        m["xT"] = xcore
        m["cT"] = cTb
        in_maps.append(m)
    return in_maps


def gather(results):
    out = np.empty((B, N, H), np.float32)
    for ci in range(NCORES):
        b, h = divmod(ci, 2)
        out[b, h * TOK:(h + 1) * TOK, :] = results[ci]["outT"].T
    return out


def get_nc(taps=()):
    key = ("nc", tuple(sorted(taps)))
    if key not in _CACHE:
        _CACHE[key] = _build(taps=taps)
    return _CACHE[key]


def kernel(**inputs):
    from concourse import bass_utils
    nc = get_nc()
    in_maps = make_in_maps(**inputs)
    res = bass_utils.run_bass_kernel_spmd(nc, in_maps,
                                          core_ids=list(range(NCORES)))
    return gather(res.results)


# revision 17
# speedup vs baseline: 1.2918x; 1.0954x over previous
"""DiT block kernel for Trainium2, 8 NeuronCores.

Sharding: core ci = (batch b = ci//2, token-half h = ci%2). Each core computes
the full DiT block for 512 "q" tokens of one batch. Attention needs all 1024
k/v tokens of the batch, so the per-batch LN/qkv-kv work is duplicated across
the 2 cores sharing a batch (≈14% FLOP overhead, zero collectives).

Everything on-device runs in a "transposed" activation layout [feature, token]
so that per-feature modulation vectors (adaLN shift/scale/gate) and all biases
are per-partition scalars, and all matmuls consume weights in their natural
or host-pre-tiled layout with zero on-device transposes of big tensors.

Key layout identities (out = lhsT.T @ rhs on the PE):
  cm       [1,6H]    : lhsT = silu(c).T k-slabs,  rhs = w_mod slabs
  q/kT     [C, tok]  : lhsT = w_qkv tiles,        rhs = xmT
  v        [tok, C]  : lhsT = xmT,                rhs = w_v slabs
  scoresT  [kt, qt]  : lhsT = kT head slice,      rhs = qT head slice (K=64)
  att_avT  [65, qt]  : lhsT = [v_head | ones],    rhs = exp(scoresT)  (row 64
                       accumulates the softmax denominator for free)
  yT       [H, tok]  : lhsT = w_proj tiles,       rhs = att_outT
  hidT     [dff,tok] : lhsT = w1 tiles,           rhs = xm2T
  finalT   [H, tok]  : lhsT = w2 tiles (bf16),    rhs = hidT (bf16)

LN stats (mean / mean-of-squares over the H partition dim) are computed with
ones-vector matmuls on the PE; rstd = exp(-0.5*ln(var+eps)) on ACT (keeps the
Ln/Exp table set hot, avoids the banned Rsqrt).
"""

import sys

for _p in ("/opt/trn_rl_repo",):
    if _p not in sys.path:
        sys.path.insert(0, _p)

import numpy as np
import ml_dtypes

B, N, H = 4, 1024, 1024
NH, D = 16, 64
DFF = 4 * H
EPS = 1e-5
NCORES = 8
TOK = N // 2      # q tokens per core
KT = H // 128     # 8  H-tiles
MT_QK = 16        # q+k column tiles
MT_FF = DFF // 128  # 32
VA = NH * (D + 1)   # 1040 v_aug columns

_CACHE = {}


def _build(taps=()):
    import concourse.bacc as bacc
    import concourse.tile as tile
    import concourse.mybir as mybir
    from concourse.masks import make_identity

    f32 = mybir.dt.float32
    f32r = mybir.dt.float32r
    bf16 = mybir.dt.bfloat16
    AF = mybir.ActivationFunctionType
    OP = mybir.AluOpType

    nc = bacc.Bacc("TRN2", target_bir_lowering=False, debug=False,
                   num_devices=NCORES)

    dxT = nc.dram_tensor("xT", [H, N], f32, kind="ExternalInput")
    dcT = nc.dram_tensor("cT", [128, KT], f32, kind="ExternalInput")
    dwmod = nc.dram_tensor("wmod", [12, 128, KT, 512], bf16, kind="ExternalInput")
    dbmodT = nc.dram_tensor("bmodT", [128, 48], f32, kind="ExternalInput")
    dwqk = nc.dram_tensor("wqk", [MT_QK, 128, KT, 128], bf16, kind="ExternalInput")
    dbqkT = nc.dram_tensor("bqkT", [128, MT_QK], f32, kind="ExternalInput")
    dwv = nc.dram_tensor("wv", [H, H], bf16, kind="ExternalInput")
    dbvT = nc.dram_tensor("bvT", [128, KT], f32, kind="ExternalInput")
    dwproj = nc.dram_tensor("wproj", [KT, 128, KT, 128], bf16, kind="ExternalInput")
    dbprojT = nc.dram_tensor("bprojT", [128, KT], f32, kind="ExternalInput")
    dw1 = nc.dram_tensor("w1", [MT_FF, 128, KT, 128], bf16, kind="ExternalInput")
    db1T = nc.dram_tensor("b1T", [128, MT_FF], f32, kind="ExternalInput")
    dw2 = nc.dram_tensor("w2", [KT, 128, MT_FF, 128], bf16, kind="ExternalInput")
    db2T = nc.dram_tensor("b2T", [128, KT], f32, kind="ExternalInput")
    doutT = nc.dram_tensor("outT", [H, TOK], f32, kind="ExternalOutput")

    _tapt = {}

    def tap(name, ap, shape=None, dtype=None):
        if name not in taps:
            return
        if name not in _tapt:
            _tapt[name] = nc.dram_tensor(
                "tap_" + name, list(shape or ap.shape), dtype or ap.dtype,
                kind="ExternalOutput")
        nc.sync.dma_start(out=_tapt[name].ap(), in_=ap)

    def r(ap):
        return ap.bitcast(f32r)

    with tile.TileContext(nc) as tc:
        from contextlib import ExitStack
        es = ExitStack()
        with es:
            pconst = es.enter_context(tc.tile_pool(name="const", bufs=1))
            big = es.enter_context(tc.tile_pool(name="big", bufs=1))
            pmm = es.enter_context(tc.tile_pool(name="pmm", bufs=2, space="PSUM"))
            psc = es.enter_context(tc.tile_pool(name="psc", bufs=2, space="PSUM"))
            pav = es.enter_context(tc.tile_pool(name="pav", bufs=2, space="PSUM"))

            # ---- constants ----
            ident = pconst.tile([128, 128], f32)
            make_identity(nc, ident)
            ones = pconst.tile([128, 1], f32)
            nc.vector.memset(ones[:].bitcast(mybir.dt.uint32), 0x3F800000)
            ones_bf = pconst.tile([128, 1], bf16)
            nc.vector.memset(ones_bf, 1.0)

            cT_sb = pconst.tile([128, KT], f32)
            nc.sync.dma_start(out=cT_sb, in_=dcT.ap())
            bmodT = pconst.tile([128, 48], f32)
            nc.sync.dma_start(out=bmodT, in_=dbmodT.ap())
            bqkT = pconst.tile([128, MT_QK], f32)
            nc.sync.dma_start(out=bqkT, in_=dbqkT.ap())
            bvT = pconst.tile([128, KT], f32)
            nc.sync.dma_start(out=bvT, in_=dbvT.ap())
            bprojT = pconst.tile([128, KT], f32)
            nc.sync.dma_start(out=bprojT, in_=dbprojT.ap())
            b1T = pconst.tile([128, MT_FF], f32)
            nc.sync.dma_start(out=b1T, in_=db1T.ap())
            b2T = pconst.tile([128, KT], f32)
            nc.sync.dma_start(out=b2T, in_=db2T.ap())

            silucT = pconst.tile([128, KT], bf16)
            nc.scalar.activation(out=silucT, in_=cT_sb, func=AF.Silu)
            eps_sb = pconst.tile([1, 1], f32)
            nc.vector.memset(eps_sb, EPS)

            # ---- xT load ----
            xT = big.tile([128, KT, N], f32, tag="A")
            for k in range(KT):
                nc.sync.dma_start(out=xT[:, k, :].bitcast(f32r),
                                  in_=dxT.ap()[k * 128:(k + 1) * 128, :].bitcast(f32r))

            # ---- cm = silu(c) @ w_mod ; staged via DRAM into [48, 128] ----
            cm_rs = pconst.tile([48, 128], f32)
            with tc.tile_pool(name="pwmod", bufs=3) as pwmod, \
                 tc.tile_pool(name="pcmb", bufs=2) as pcmb, \
                 tc.tile_pool(name="pcmd", bufs=1, space="DRAM") as pcmd:
                cmdram = pcmd.tile([12, 512], f32)
                for grp in range(3):
                    ps = pmm.tile([128, 512], f32, tag="mm")
                    wts = []
                    for j in range(4):
                        nchk = grp * 4 + j
                        wt = pwmod.tile([128, KT, 512], bf16, tag="wmod",
                                        name=f"wmod{nchk}")
                        nc.sync.dma_start(out=wt, in_=dwmod.ap()[nchk])
                        wts.append(wt)
                    for j in range(4):
                        for k in range(KT):
                            nc.tensor.matmul(
                                ps[32 * j:32 * j + 1, :], silucT[:, k:k + 1],
                                wts[j][:, k, :], tile_position=(0, 32 * j),
                                start=(k == 0), stop=(k == KT - 1))
                    for j in range(4):
                        nchk = grp * 4 + j
                        cb = pcmb.tile([1, 512], f32, tag="cmb")
                        nc.any.tensor_copy(out=cb, in_=ps[32 * j:32 * j + 1, :])
                        nc.sync.dma_start(out=cmdram[nchk:nchk + 1, :], in_=cb[:])
                nc.sync.dma_start(
                    out=cm_rs,
                    in_=cmdram[:].rearrange("a (b f) -> (a b) f", b=4))
            tap("cm_rs", cm_rs[:])
            ps_cmT = pmm.tile([128, 48], f32, tag="mm")
            nc.tensor.transpose(ps_cmT, cm_rs[:], ident[0:48, 0:48])
            cmT = pconst.tile([128, 48], f32)
            nc.vector.tensor_add(out=cmT, in0=ps_cmT[:], in1=bmodT[:])
            tap("cmT_raw", cmT[:])
            # cmT columns: v*8+t ; v: 0 sh_msa, 1 sc_msa, 2 g_msa, 3 sh_mlp,
            # 4 sc_mlp, 5 g_mlp
            sc1p = pconst.tile([128, KT], f32)
            nc.vector.tensor_scalar_add(out=sc1p, in0=cmT[:, 8:16], scalar1=1.0)
            g1p = pconst.tile([128, KT], f32)
            nc.vector.tensor_scalar_add(out=g1p, in0=cmT[:, 16:24], scalar1=1.0)
            sc2p = pconst.tile([128, KT], f32)
            nc.vector.tensor_scalar_add(out=sc2p, in0=cmT[:, 32:40], scalar1=1.0)
            tap("cmT", cmT[:])

            # ---- LN1 stats over H (partition dim) via ones-matmuls ----
            pmub_cm = tc.tile_pool(name="pmub", bufs=1)
            pmub = pmub_cm.__enter__()
            muB = pmub.tile([128, N], f32)
            rstdB = pmub.tile([128, N], f32)
            pstat_cm = tc.tile_pool(name="pstat", bufs=1)
            pstat = pstat_cm.__enter__()
            st_mu = pstat.tile([1, N], f32)
            st_e2 = pstat.tile([1, N], f32)
            st_var = pstat.tile([1, N], f32)
            st_rstd = pstat.tile([1, N], f32)

            with tc.tile_pool(name="pxsq", bufs=3) as pxsq:
                ps = pmm.tile([128, 512], f32, tag="mm")
                for nch in range(2):
                    for k in range(KT):
                        xsq = pxsq.tile([128, 512], bf16, tag="xsq")
                        nc.vector.tensor_mul(
                            out=xsq, in0=xT[:, k, nch * 512:(nch + 1) * 512],
                            in1=xT[:, k, nch * 512:(nch + 1) * 512])
                        nc.tensor.matmul(ps[64 + 32 * nch:64 + 32 * nch + 1, :],
                                         ones_bf[:], xsq[:],
                                         tile_position=(0, 64 + 32 * nch),
                                         start=(k == 0), stop=(k == KT - 1))
                # mean via bf16 copies of xT (tile_position needs bf16 path)
                for nch in range(2):
                    for k in range(KT):
                        xb = pxsq.tile([128, 512], bf16, tag="xb")
                        nc.vector.tensor_copy(
                            out=xb, in_=xT[:, k, nch * 512:(nch + 1) * 512])
                        nc.tensor.matmul(ps[32 * nch:32 * nch + 1, :],
                                         ones_bf[:], xb[:],
                                         tile_position=(0, 32 * nch),
                                         start=(k == 0), stop=(k == KT - 1))
                for nch in range(2):
                    nc.scalar.activation(out=st_mu[0:1, nch * 512:(nch + 1) * 512],
                                         in_=ps[32 * nch:32 * nch + 1, :],
                                         func=AF.Copy, scale=1.0 / H)
                    nc.scalar.activation(out=st_e2[0:1, nch * 512:(nch + 1) * 512],
                                         in_=ps[64 + 32 * nch:64 + 32 * nch + 1, :],
                                         func=AF.Copy, scale=1.0 / H)
            nc.vector.tensor_mul(out=st_var, in0=st_mu, in1=st_mu)
            nc.vector.tensor_sub(out=st_var, in0=st_e2, in1=st_var)
            # rstd = exp(-0.5 * ln(var + eps))
            nc.scalar.activation(out=st_rstd, in_=st_var, func=AF.Ln, bias=eps_sb[:])
            nc.scalar.activation(out=st_rstd, in_=st_rstd, func=AF.Exp, scale=-0.5)
            tap("st_mu", st_mu[:])
            tap("st_rstd", st_rstd[:])
            nc.gpsimd.partition_broadcast(muB[:], st_mu[:])
            nc.gpsimd.partition_broadcast(rstdB[:], st_rstd[:])
            pstat_cm.__exit__(None, None, None)

            # ---- modulate: xmT = ((xT - mu) * rstd) * (1+sc_msa) + sh_msa ----
            xmT = big.tile([128, KT, N], bf16, tag="B")
            with tc.tile_pool(name="ptmp", bufs=2) as ptmp:
                for k in range(KT):
                    tmp = ptmp.tile([128, N], f32, tag="tmp")
                    nc.vector.tensor_sub(out=tmp, in0=xT[:, k, :], in1=muB[:])
                    nc.vector.tensor_mul(out=tmp, in0=tmp, in1=rstdB[:])
                    nc.scalar.activation(out=xmT[:, k, :], in_=tmp,
                                         func=AF.Identity,
                                         scale=sc1p[:, k:k + 1],
                                         bias=cmT[:, k:k + 1])

            tap("xmT0", xmT[:, 0, :])
            pmub_cm.__exit__(None, None, None)

            # ---- qT / kT ----
            qT = big.tile([128, KT, TOK], bf16, tag="D")
            kTt = big.tile([128, KT, N], bf16, tag="A")
            with tc.tile_pool(name="pwqk", bufs=3) as pwqk:
                for mt in range(MT_QK):
                    wt = pwqk.tile([128, KT, 128], bf16, tag="wqk")
                    nc.sync.dma_start(out=wt, in_=dwqk.ap()[mt])
                    is_q = mt < KT
                    for nch in ([0] if is_q else [0, 1]):
                        ps = pmm.tile([128, 512], f32, tag="mm")
                        for k in range(KT):
                            nc.tensor.matmul(
                                ps, wt[:, k, :],
                                xmT[:, k, nch * 512:(nch + 1) * 512],
                                start=(k == 0), stop=(k == KT - 1))
                        if is_q:
                            dst = qT[:, mt, :]
                        else:
                            dst = kTt[:, mt - KT, nch * 512:(nch + 1) * 512]
                        nc.vector.tensor_scalar(
                            out=dst, in0=ps[:],
                            scalar1=bqkT[:, mt:mt + 1],
                            scalar2=None, op0=OP.add)

            tap("qT0", qT[:, 0, :])
            tap("kT0", kTt[:, 0, 0:512])

            # ---- v (natural layout, augmented with ones column per head) ----
            v_aug = big.tile([128, KT, VA], bf16, tag="C")
            nc.vector.memset(
                v_aug[:].rearrange("p k (h c) -> p k h c", c=D + 1)[:, :, :, D:D + 1],
                1.0)
            with tc.tile_pool(name="pwv", bufs=1) as pwv:
                for nch in range(2):
                    # half-slabs of w_v for this output-column chunk
                    wv_slabs = []
                    for k in range(KT):
                        wv = pwv.tile([128, 512], bf16, tag=f"wv{k}")
                        nc.sync.dma_start(
                            out=wv,
                            in_=dwv.ap()[k * 128:(k + 1) * 128,
                                         nch * 512:(nch + 1) * 512])
                        wv_slabs.append(wv)
                    for mtok in range(KT):
                        ps = pmm.tile([128, 512], f32, tag="mm")
                        for k in range(KT):
                            nc.tensor.matmul(
                                ps, xmT[:, k, mtok * 128:(mtok + 1) * 128],
                                wv_slabs[k][:],
                                start=(k == 0), stop=(k == KT - 1))
                        for h8 in range(8):
                            hh = nch * 8 + h8
                            nc.any.tensor_copy(
                                out=v_aug[:, mtok, hh * (D + 1):hh * (D + 1) + D],
                                in_=ps[:, h8 * D:(h8 + 1) * D])

            tap("vaug0", v_aug[:, 0, :])

            # ---- attention: per head pair ----
            att = big.tile([128, KT, TOK], bf16, tag="E")
            with tc.tile_pool(name="pexp", bufs=4) as pexp, \
                 tc.tile_pool(name="pnorm", bufs=3) as pnorm:
                for t in range(KT):
                    psA = pav.tile([D + 1, 512], f32, tag="av")
                    psB = pav.tile([D + 1, 512], f32, tag="av")
                    for kb in range(KT):
                        pss = psc.tile([128, 1024], f32, tag="sc")
                        nc.tensor.matmul(
                            pss[:, 0:512],
                            kTt[0:64, t, kb * 128:(kb + 1) * 128],
                            qT[0:64, t, :], start=True, stop=True)
                        nc.tensor.matmul(
                            pss[:, 512:1024],
                            kTt[64:128, t, kb * 128:(kb + 1) * 128],
                            qT[64:128, t, :], start=True, stop=True)
                        ex = pexp.tile([128, 1024], bf16, tag="exp")
                        nc.scalar.activation(out=ex, in_=pss[:],
                                             func=AF.Exp, scale=1.0 / np.sqrt(D))
                        hA, hB = 2 * t, 2 * t + 1
                        nc.tensor.matmul(
                            psA, v_aug[:, kb, hA * (D + 1):(hA + 1) * (D + 1)],
                            ex[:, 0:512],
                            start=(kb == 0), stop=(kb == KT - 1))
                        nc.tensor.matmul(
                            psB, v_aug[:, kb, hB * (D + 1):(hB + 1) * (D + 1)],
                            ex[:, 512:1024],
                            start=(kb == 0), stop=(kb == KT - 1))
                    # stage unnormalized att to SBUF immediately (frees the
                    # av psum fast) and run the recip chain off the psum path
                    rbs = []
                    us = []
                    for psX in (psA, psB):
                        rx = pnorm.tile([1, TOK], f32, tag="rx")
                        nc.scalar.activation(out=rx, in_=psX[D:D + 1, :], func=AF.Ln)
                        nc.scalar.activation(out=rx, in_=rx, func=AF.Exp, scale=-1.0)
                        rb = pnorm.tile([64, TOK], f32, tag="rb")
                        nc.gpsimd.partition_broadcast(rb[:], rx[:])
                        rbs.append(rb)
                        attu = pnorm.tile([64, TOK], f32, tag="attu")
                        nc.vector.tensor_copy(out=attu, in_=psX[0:D, :])
                        us.append(attu)
                    nc.vector.tensor_mul(out=att[0:64, t, :],
                                         in0=us[0][:], in1=rbs[0][:])
                    nc.vector.tensor_mul(out=att[64:128, t, :],
                                         in0=us[1][:], in1=rbs[1][:])
                    nc.scalar.activation(out=att[:, t, :],
                                         in_=att[:, t, :],
                                         func=AF.Identity, bias=bvT[:, t:t + 1])

            tap("att0", att[:, 0, :])
            tap("att7", att[:, 7, :])

            # ---- proj -> yT = (proj + b_proj) * (1 + gate_msa) ----
            yT = big.tile([128, KT, TOK], f32, tag="F")
            with tc.tile_pool(name="pwproj", bufs=2) as pwproj:
                for mt in range(KT):
                    wt = pwproj.tile([128, KT, 128], bf16, tag="wproj")
                    nc.sync.dma_start(out=wt, in_=dwproj.ap()[mt])
                    ps = pmm.tile([128, 512], f32, tag="mm")
                    for k in range(KT):
                        nc.tensor.matmul(ps, wt[:, k, :], att[:, k, :],
                                         start=(k == 0), stop=(k == KT - 1))
                    nc.vector.tensor_scalar(
                        out=yT[:, mt, :].bitcast(f32r), in0=ps[:],
                        scalar1=bprojT[:, mt:mt + 1], scalar2=g1p[:, mt:mt + 1],
                        op0=OP.add, op1=OP.mult)  # f32r-rounded for LN2 mean mm

            # ---- LN2 stats + modulate -> xm2T ----
            pmub2_cm = tc.tile_pool(name="pmub2", bufs=1)
            pmub2 = pmub2_cm.__enter__()
            muB2 = pmub2.tile([128, TOK], f32)
            rstdB2 = pmub2.tile([128, TOK], f32)
            pstat2_cm = tc.tile_pool(name="pstat2", bufs=1)
            pstat2 = pstat2_cm.__enter__()
            st2_mu = pstat2.tile([1, TOK], f32)
            st2_e2 = pstat2.tile([1, TOK], f32)
            st2_var = pstat2.tile([1, TOK], f32)
            st2_rstd = pstat2.tile([1, TOK], f32)
            with tc.tile_pool(name="pysq", bufs=2) as pysq:
                ps = pmm.tile([128, 512], f32, tag="mm")
                for k in range(KT):
                    yb = pysq.tile([128, TOK], bf16, tag="yb")
                    nc.vector.tensor_copy(out=yb, in_=yT[:, k, :])
                    nc.tensor.matmul(ps[0:1, :], ones_bf[:], yb[:],
                                     tile_position=(0, 0),
                                     start=(k == 0), stop=(k == KT - 1))
                for k in range(KT):
                    ysq = pysq.tile([128, TOK], bf16, tag="ysq")
                    nc.vector.tensor_mul(out=ysq, in0=yT[:, k, :], in1=yT[:, k, :])
                    nc.tensor.matmul(ps[32:33, :], ones_bf[:], ysq[:],
                                     tile_position=(0, 32),
                                     start=(k == 0), stop=(k == KT - 1))
                nc.scalar.activation(out=st2_mu, in_=ps[0:1, :], func=AF.Copy,
                                     scale=1.0 / H)
                nc.scalar.activation(out=st2_e2, in_=ps[32:33, :], func=AF.Copy,
                                     scale=1.0 / H)
            nc.vector.tensor_mul(out=st2_var, in0=st2_mu, in1=st2_mu)
            nc.vector.tensor_sub(out=st2_var, in0=st2_e2, in1=st2_var)
            nc.scalar.activation(out=st2_rstd, in_=st2_var, func=AF.Ln, bias=eps_sb[:])
            nc.scalar.activation(out=st2_rstd, in_=st2_rstd, func=AF.Exp, scale=-0.5)
            nc.gpsimd.partition_broadcast(muB2[:], st2_mu[:])
            nc.gpsimd.partition_broadcast(rstdB2[:], st2_rstd[:])
            pstat2_cm.__exit__(None, None, None)

            xm2T = big.tile([128, KT, TOK], bf16, tag="D")
            with tc.tile_pool(name="ptmp2", bufs=2) as ptmp2:
                for k in range(KT):
                    tmp = ptmp2.tile([128, TOK], f32, tag="tmp2")
                    nc.vector.tensor_sub(out=tmp, in0=yT[:, k, :], in1=muB2[:])
                    nc.vector.tensor_mul(out=tmp, in0=tmp, in1=rstdB2[:])
                    nc.scalar.activation(out=xm2T[:, k, :], in_=tmp,
                                         func=AF.Identity,
                                         scale=sc2p[:, k:k + 1],
                                         bias=cmT[:, 24 + k:24 + k + 1])

            tap("yT0", yT[:, 0, :])
            tap("xm2T0", xm2T[:, 0, :])
            pmub2_cm.__exit__(None, None, None)

            # ---- mlp1: hidT = gelu(w1T @ xm2T + b1) (bf16) ----
            hidT = big.tile([128, MT_FF, TOK], bf16, tag="B")
            with tc.tile_pool(name="pw1", bufs=3) as pw1:
                for mt in range(MT_FF):
                    wt = pw1.tile([128, KT, 128], bf16, tag="w1")
                    nc.sync.dma_start(out=wt, in_=dw1.ap()[mt])
                    ps = pmm.tile([128, 512], f32, tag="mm")
                    for k in range(KT):
                        nc.tensor.matmul(ps, wt[:, k, :], xm2T[:, k, :],
                                         start=(k == 0), stop=(k == KT - 1))
                    nc.scalar.activation(out=hidT[:, mt, :], in_=ps[:],
                                         func=AF.Gelu, bias=b1T[:, mt:mt + 1])

            tap("hid0", hidT[:, 0, :])

            # ---- mlp2 + residual -> out ----
            with tc.tile_pool(name="pw2", bufs=2) as pw2, \
                 tc.tile_pool(name="pout", bufs=3) as pout:
                for mt in range(KT):
                    wt = pw2.tile([128, MT_FF, 128], bf16, tag="w2")
                    nc.sync.dma_start(out=wt, in_=dw2.ap()[mt])
                    ps = pmm.tile([128, 512], f32, tag="mm")
                    for kk in range(MT_FF):
                        nc.tensor.matmul(ps, wt[:, kk, :], hidT[:, kk, :],
                                         start=(kk == 0), stop=(kk == MT_FF - 1))
                    ot = pout.tile([128, TOK], f32, tag="out")
                    nc.vector.tensor_scalar(
                        out=ot, in0=ps[:], scalar1=b2T[:, mt:mt + 1],
                        scalar2=cmT[:, 40 + mt:40 + mt + 1],
                        op0=OP.add, op1=OP.mult)
                    nc.vector.tensor_add(out=ot, in0=ot, in1=yT[:, mt, :])
                    nc.sync.dma_start(out=doutT.ap()[mt * 128:(mt + 1) * 128, :],
                                      in_=ot)

    nc.compile()
    return nc


def _prep_shared(w_mod, b_mod, w_qkv, b_qkv, w_proj, b_proj, w1, b1, w2, b2):
    c32 = np.ascontiguousarray
    f32 = np.float32
    shared = {
        "wmod": c32(w_mod.reshape(8, 128, 12, 512).transpose(2, 1, 0, 3)).astype(ml_dtypes.bfloat16),
        "bmodT": c32(b_mod.reshape(48, 128).T).astype(f32, copy=False),
        "wqk": c32(w_qkv[:, :2048].reshape(8, 128, 16, 128).transpose(2, 1, 0, 3)).astype(ml_dtypes.bfloat16),
        "bqkT": c32(b_qkv[:2048].reshape(16, 128).T).astype(f32, copy=False),
        "wv": c32(w_qkv[:, 2048:]).astype(ml_dtypes.bfloat16),
        "bvT": c32(b_qkv[2048:].reshape(8, 128).T).astype(f32, copy=False),
        "wproj": c32(w_proj.reshape(8, 128, 8, 128).transpose(2, 1, 0, 3)).astype(ml_dtypes.bfloat16),
        "bprojT": c32(b_proj.reshape(8, 128).T).astype(f32, copy=False),
        "w1": c32(w1.reshape(8, 128, 32, 128).transpose(2, 1, 0, 3)).astype(ml_dtypes.bfloat16),
        "b1T": c32(b1.reshape(32, 128).T).astype(f32, copy=False),
        "w2": c32(w2.reshape(32, 128, 8, 128).transpose(2, 1, 0, 3)).astype(ml_dtypes.bfloat16),
        "b2T": c32(b2.reshape(8, 128).T).astype(f32, copy=False),
    }
    return shared


def make_in_maps(x, c, w_mod, b_mod, w_qkv, b_qkv, w_proj, b_proj, w1, b1, w2, b2):
    x = np.asarray(x, np.float32)
    c = np.asarray(c, np.float32)
    shared = _prep_shared(np.asarray(w_mod, np.float32), np.asarray(b_mod, np.float32),
                          np.asarray(w_qkv, np.float32), np.asarray(b_qkv, np.float32),
                          np.asarray(w_proj, np.float32), np.asarray(b_proj, np.float32),
                          np.asarray(w1, np.float32), np.asarray(b1, np.float32),
                          np.asarray(w2, np.float32), np.asarray(b2, np.float32))
    in_maps = []
    for ci in range(NCORES):
        b, h = divmod(ci, 2)
        xTb = x[b].T
        xcore = np.ascontiguousarray(
            np.concatenate([xTb[:, h * TOK:(h + 1) * TOK],
                            xTb[:, (1 - h) * TOK:(2 - h) * TOK]], axis=1))
        cTb = np.ascontiguousarray(c[b].reshape(8, 128).T)
        m = dict(shared)
        m["xT"] = xcore
        m["cT"] = cTb
        in_maps.append(m)
    return in_maps


def gather(results):
    out = np.empty((B, N, H), np.float32)
    for ci in range(NCORES):
        b, h = divmod(ci, 2)
        out[b, h * TOK:(h + 1) * TOK, :] = results[ci]["outT"].T
    return out


def get_nc(taps=()):
    key = ("nc", tuple(sorted(taps)))
    if key not in _CACHE:
        _CACHE[key] = _build(taps=taps)
    return _CACHE[key]


def kernel(**inputs):
    from concourse import bass_utils
    nc = get_nc()
    in_maps = make_in_maps(**inputs)
    res = bass_utils.run_bass_kernel_spmd(nc, in_maps,
                                          core_ids=list(range(NCORES)))
    return gather(res.results)


# revision 23
# speedup vs baseline: 1.2931x; 1.0010x over previous
"""DiT block kernel for Trainium2, 8 NeuronCores.

Sharding: core ci = (batch b = ci//2, token-half h = ci%2). Each core computes
the full DiT block for 512 "q" tokens of one batch. Attention needs all 1024
k/v tokens of the batch, so the per-batch LN/qkv-kv work is duplicated across
the 2 cores sharing a batch (≈14% FLOP overhead, zero collectives).

Everything on-device runs in a "transposed" activation layout [feature, token]
so that per-feature modulation vectors (adaLN shift/scale/gate) and all biases
are per-partition scalars, and all matmuls consume weights in their natural
or host-pre-tiled layout with zero on-device transposes of big tensors.

Key layout identities (out = lhsT.T @ rhs on the PE):
  cm       [1,6H]    : lhsT = silu(c).T k-slabs,  rhs = w_mod slabs
  q/kT     [C, tok]  : lhsT = w_qkv tiles,        rhs = xmT
  v        [tok, C]  : lhsT = xmT,                rhs = w_v slabs
  scoresT  [kt, qt]  : lhsT = kT head slice,      rhs = qT head slice (K=64)
  att_avT  [65, qt]  : lhsT = [v_head | ones],    rhs = exp(scoresT)  (row 64
                       accumulates the softmax denominator for free)
  yT       [H, tok]  : lhsT = w_proj tiles,       rhs = att_outT
  hidT     [dff,tok] : lhsT = w1 tiles,           rhs = xm2T
  finalT   [H, tok]  : lhsT = w2 tiles (bf16),    rhs = hidT (bf16)

LN stats (mean / mean-of-squares over the H partition dim) are computed with
ones-vector matmuls on the PE; rstd = exp(-0.5*ln(var+eps)) on ACT (keeps the
Ln/Exp table set hot, avoids the banned Rsqrt).
"""

import sys

for _p in ("/opt/trn_rl_repo",):
    if _p not in sys.path:
        sys.path.insert(0, _p)

import numpy as np
import ml_dtypes

B, N, H = 4, 1024, 1024
NH, D = 16, 64
DFF = 4 * H
EPS = 1e-5
NCORES = 8
TOK = N // 2      # q tokens per core
KT = H // 128     # 8  H-tiles
MT_QK = 16        # q+k column tiles
MT_FF = DFF // 128  # 32
VA = NH * (D + 1)   # 1040 v_aug columns

_CACHE = {}


def _build(taps=()):
    import concourse.bacc as bacc
    import concourse.tile as tile
    import concourse.mybir as mybir
    from concourse.masks import make_identity

    f32 = mybir.dt.float32
    f32r = mybir.dt.float32r
    bf16 = mybir.dt.bfloat16
    AF = mybir.ActivationFunctionType
    OP = mybir.AluOpType

    nc = bacc.Bacc("TRN2", target_bir_lowering=False, debug=False,
                   num_devices=NCORES)

    dxT = nc.dram_tensor("xT", [H, N], f32, kind="ExternalInput")
    dcT = nc.dram_tensor("cT", [128, KT], f32, kind="ExternalInput")
    dwmod = nc.dram_tensor("wmod", [12, 128, KT, 512], bf16, kind="ExternalInput")
    dbmodT = nc.dram_tensor("bmodT", [128, 48], f32, kind="ExternalInput")
    dwqk = nc.dram_tensor("wqk", [MT_QK, 128, KT, 128], bf16, kind="ExternalInput")
    dbqkT = nc.dram_tensor("bqkT", [128, MT_QK], f32, kind="ExternalInput")
    dwv = nc.dram_tensor("wv", [H, H], bf16, kind="ExternalInput")
    dbvT = nc.dram_tensor("bvT", [128, KT], f32, kind="ExternalInput")
    dwproj = nc.dram_tensor("wproj", [KT, 128, KT, 128], bf16, kind="ExternalInput")
    dbprojT = nc.dram_tensor("bprojT", [128, KT], f32, kind="ExternalInput")
    dw1 = nc.dram_tensor("w1", [MT_FF, 128, KT, 128], bf16, kind="ExternalInput")
    db1T = nc.dram_tensor("b1T", [128, MT_FF], f32, kind="ExternalInput")
    dw2 = nc.dram_tensor("w2", [KT, 128, MT_FF, 128], bf16, kind="ExternalInput")
    db2T = nc.dram_tensor("b2T", [128, KT], f32, kind="ExternalInput")
    doutT = nc.dram_tensor("outT", [H, TOK], f32, kind="ExternalOutput")

    _tapt = {}

    def tap(name, ap, shape=None, dtype=None):
        if name not in taps:
            return
        if name not in _tapt:
            _tapt[name] = nc.dram_tensor(
                "tap_" + name, list(shape or ap.shape), dtype or ap.dtype,
                kind="ExternalOutput")
        nc.sync.dma_start(out=_tapt[name].ap(), in_=ap)

    def r(ap):
        return ap.bitcast(f32r)

    with tile.TileContext(nc) as tc:
        from contextlib import ExitStack
        es = ExitStack()
        with es:
            pconst = es.enter_context(tc.tile_pool(name="const", bufs=1))
            big = es.enter_context(tc.tile_pool(name="big", bufs=1))
            pmm = es.enter_context(tc.tile_pool(name="pmm", bufs=2, space="PSUM"))
            psc = es.enter_context(tc.tile_pool(name="psc", bufs=2, space="PSUM"))
            pav = es.enter_context(tc.tile_pool(name="pav", bufs=2, space="PSUM"))

            # ---- constants ----
            ident = pconst.tile([128, 128], f32)
            make_identity(nc, ident)
            ones = pconst.tile([128, 1], f32)
            nc.vector.memset(ones[:].bitcast(mybir.dt.uint32), 0x3F800000)
            ones_bf = pconst.tile([128, 1], bf16)
            nc.vector.memset(ones_bf, 1.0)
            ones_row = pconst.tile([1, 128], f32)
            nc.vector.memset(ones_row[:].bitcast(mybir.dt.uint32), 0x3F800000)

            cT_sb = pconst.tile([128, KT], f32)
            nc.sync.dma_start(out=cT_sb, in_=dcT.ap())
            bmodT = pconst.tile([128, 48], f32)
            nc.sync.dma_start(out=bmodT, in_=dbmodT.ap())
            bqkT = pconst.tile([128, MT_QK], f32)
            nc.sync.dma_start(out=bqkT, in_=dbqkT.ap())
            bvT = pconst.tile([128, KT], f32)
            nc.sync.dma_start(out=bvT, in_=dbvT.ap())
            bprojT = pconst.tile([128, KT], f32)
            nc.sync.dma_start(out=bprojT, in_=dbprojT.ap())
            b1T = pconst.tile([128, MT_FF], f32)
            nc.sync.dma_start(out=b1T, in_=db1T.ap())
            b2T = pconst.tile([128, KT], f32)
            nc.sync.dma_start(out=b2T, in_=db2T.ap())

            silucT = pconst.tile([128, KT], bf16)
            nc.scalar.activation(out=silucT, in_=cT_sb, func=AF.Silu)
            eps_sb = pconst.tile([1, 1], f32)
            nc.vector.memset(eps_sb, EPS)

            # ---- xT load ----
            xT = big.tile([128, KT, N], f32, tag="A")
            for k in range(KT):
                nc.sync.dma_start(out=xT[:, k, :].bitcast(f32r),
                                  in_=dxT.ap()[k * 128:(k + 1) * 128, :].bitcast(f32r))

            # ---- LN1 stats over H (partition dim) via ones-matmuls ----
            pstat_cm = tc.tile_pool(name="pstat", bufs=1)
            pstat = pstat_cm.__enter__()
            st_mu = pstat.tile([1, N], f32)
            st_e2 = pstat.tile([1, N], f32)
            st_var = pstat.tile([1, N], f32)
            st_rstd = pstat.tile([1, N], f32)

            with tc.tile_pool(name="pxsq", bufs=3) as pxsq:
                ps = pmm.tile([128, 512], f32, tag="mm")
                for nch in range(2):
                    for k in range(KT):
                        xsq = pxsq.tile([128, 512], bf16, tag="xsq")
                        nc.vector.tensor_mul(
                            out=xsq, in0=xT[:, k, nch * 512:(nch + 1) * 512],
                            in1=xT[:, k, nch * 512:(nch + 1) * 512])
                        nc.tensor.matmul(ps[64 + 32 * nch:64 + 32 * nch + 1, :],
                                         ones_bf[:], xsq[:],
                                         tile_position=(0, 64 + 32 * nch),
                                         start=(k == 0), stop=(k == KT - 1))
                # mean via bf16 copies of xT (tile_position needs bf16 path)
                for nch in range(2):
                    for k in range(KT):
                        xb = pxsq.tile([128, 512], bf16, tag="xb")
                        nc.vector.tensor_copy(
                            out=xb, in_=xT[:, k, nch * 512:(nch + 1) * 512])
                        nc.tensor.matmul(ps[32 * nch:32 * nch + 1, :],
                                         ones_bf[:], xb[:],
                                         tile_position=(0, 32 * nch),
                                         start=(k == 0), stop=(k == KT - 1))
                for nch in range(2):
                    nc.scalar.activation(out=st_mu[0:1, nch * 512:(nch + 1) * 512]
                                         .bitcast(f32r),
                                         in_=ps[32 * nch:32 * nch + 1, :],
                                         func=AF.Copy, scale=1.0 / H)
                    nc.scalar.activation(out=st_e2[0:1, nch * 512:(nch + 1) * 512],
                                         in_=ps[64 + 32 * nch:64 + 32 * nch + 1, :],
                                         func=AF.Copy, scale=1.0 / H)
            nc.vector.tensor_mul(out=st_var, in0=st_mu, in1=st_mu)
            nc.vector.tensor_sub(out=st_var, in0=st_e2, in1=st_var)
            # rstd = exp(-0.5 * ln(var + eps))
            nc.scalar.activation(out=st_var, in_=st_var, func=AF.Ln, bias=eps_sb[:])
            nc.scalar.activation(out=st_rstd[:].bitcast(f32r), in_=st_var,
                                 func=AF.Exp, scale=-0.5)
            tap("st_mu", st_mu[:])
            tap("st_rstd", st_rstd[:])
            muB = psc.tile([128, N], f32, tag="sc", name="muB")
            rstdB = psc.tile([128, N], f32, tag="sc", name="rstdB")
            for nch in range(2):
                nc.tensor.matmul(muB[:, nch * 512:(nch + 1) * 512],
                                 ones_row[:].bitcast(f32r),
                                 st_mu[0:1, nch * 512:(nch + 1) * 512].bitcast(f32r),
                                 start=True, stop=True)
                nc.tensor.matmul(rstdB[:, nch * 512:(nch + 1) * 512],
                                 ones_row[:].bitcast(f32r),
                                 st_rstd[0:1, nch * 512:(nch + 1) * 512].bitcast(f32r),
                                 start=True, stop=True)
            pstat_cm.__exit__(None, None, None)

            # ---- cm = silu(c) @ w_mod ; staged via DRAM into [48, 128] ----
            cm_rs = pconst.tile([48, 128], f32)
            with tc.tile_pool(name="pwmod", bufs=3) as pwmod, \
                 tc.tile_pool(name="pcmb", bufs=2) as pcmb, \
                 tc.tile_pool(name="pcmd", bufs=1, space="DRAM") as pcmd:
                cmdram = pcmd.tile([12, 512], f32)
                for grp in range(3):
                    ps = pmm.tile([128, 512], f32, tag="mm")
                    wts = []
                    for j in range(4):
                        nchk = grp * 4 + j
                        wt = pwmod.tile([128, KT, 512], bf16, tag="wmod",
                                        name=f"wmod{nchk}")
                        nc.sync.dma_start(out=wt, in_=dwmod.ap()[nchk])
                        wts.append(wt)
                    for j in range(4):
                        for k in range(KT):
                            nc.tensor.matmul(
                                ps[32 * j:32 * j + 1, :], silucT[:, k:k + 1],
                                wts[j][:, k, :], tile_position=(0, 32 * j),
                                start=(k == 0), stop=(k == KT - 1))
                    for j in range(4):
                        nchk = grp * 4 + j
                        cb = pcmb.tile([1, 512], f32, tag="cmb")
                        nc.any.tensor_copy(out=cb, in_=ps[32 * j:32 * j + 1, :])
                        nc.sync.dma_start(out=cmdram[nchk:nchk + 1, :], in_=cb[:])
                nc.sync.dma_start(
                    out=cm_rs,
                    in_=cmdram[:].rearrange("a (b f) -> (a b) f", b=4))
            tap("cm_rs", cm_rs[:])
            ps_cmT = pmm.tile([128, 48], f32, tag="mm")
            nc.tensor.transpose(ps_cmT, cm_rs[:], ident[0:48, 0:48])
            cmT = pconst.tile([128, 48], f32)
            nc.vector.tensor_add(out=cmT, in0=ps_cmT[:], in1=bmodT[:])
            tap("cmT_raw", cmT[:])
            # cmT columns: v*8+t ; v: 0 sh_msa, 1 sc_msa, 2 g_msa, 3 sh_mlp,
            # 4 sc_mlp, 5 g_mlp
            sc1p = pconst.tile([128, KT], f32)
            nc.vector.tensor_scalar_add(out=sc1p, in0=cmT[:, 8:16], scalar1=1.0)
            g1p = pconst.tile([128, KT], f32)
            nc.vector.tensor_scalar_add(out=g1p, in0=cmT[:, 16:24], scalar1=1.0)
            sc2p = pconst.tile([128, KT], f32)
            nc.vector.tensor_scalar_add(out=sc2p, in0=cmT[:, 32:40], scalar1=1.0)
            tap("cmT", cmT[:])

            # ---- modulate: xmT = ((xT - mu) * rstd) * (1+sc_msa) + sh_msa ----
            xmT = big.tile([128, KT, N], bf16, tag="B")
            with tc.tile_pool(name="ptmp", bufs=4) as ptmp:
                for k in range(KT):
                    for nch in range(2):
                        sl = slice(nch * 512, (nch + 1) * 512)
                        tmp = ptmp.tile([128, 512], f32, tag="tmp")
                        nc.vector.tensor_sub(out=tmp, in0=xT[:, k, sl],
                                             in1=muB[:, sl])
                        nc.vector.tensor_mul(out=tmp, in0=tmp, in1=rstdB[:, sl])
                        nc.scalar.activation(out=xmT[:, k, sl], in_=tmp,
                                             func=AF.Identity,
                                             scale=sc1p[:, k:k + 1],
                                             bias=cmT[:, k:k + 1])

            tap("xmT0", xmT[:, 0, :])

            # ---- qT / kT ----
            qT = big.tile([128, KT, TOK], bf16, tag="D")
            kTt = big.tile([128, KT, N], bf16, tag="A")
            with tc.tile_pool(name="pwqk", bufs=3) as pwqk:
                for mt in range(MT_QK):
                    wt = pwqk.tile([128, KT, 128], bf16, tag="wqk")
                    nc.sync.dma_start(out=wt, in_=dwqk.ap()[mt])
                    is_q = mt < KT
                    for nch in ([0] if is_q else [0, 1]):
                        ps = pmm.tile([128, 512], f32, tag="mm")
                        for k in range(KT):
                            nc.tensor.matmul(
                                ps, wt[:, k, :],
                                xmT[:, k, nch * 512:(nch + 1) * 512],
                                start=(k == 0), stop=(k == KT - 1))
                        if is_q:
                            dst = qT[:, mt, :]
                        else:
                            dst = kTt[:, mt - KT, nch * 512:(nch + 1) * 512]
                        nc.vector.tensor_scalar(
                            out=dst, in0=ps[:],
                            scalar1=bqkT[:, mt:mt + 1],
                            scalar2=None, op0=OP.add)

            tap("qT0", qT[:, 0, :])
            tap("kT0", kTt[:, 0, 0:512])

            # ---- v (natural layout, augmented with ones column per head) ----
            v_aug = big.tile([128, KT, VA], bf16, tag="C")
            nc.vector.memset(
                v_aug[:].rearrange("p k (h c) -> p k h c", c=D + 1)[:, :, :, D:D + 1],
                1.0)
            with tc.tile_pool(name="pwv", bufs=1) as pwv:
                for nch in range(2):
                    # half-slabs of w_v for this output-column chunk
                    wv_slabs = []
                    for k in range(KT):
                        wv = pwv.tile([128, 512], bf16, tag=f"wv{k}")
                        nc.sync.dma_start(
                            out=wv,
                            in_=dwv.ap()[k * 128:(k + 1) * 128,
                                         nch * 512:(nch + 1) * 512])
                        wv_slabs.append(wv)
                    for mtok in range(KT):
                        ps = pmm.tile([128, 512], f32, tag="mm")
                        for k in range(KT):
                            nc.tensor.matmul(
                                ps, xmT[:, k, mtok * 128:(mtok + 1) * 128],
                                wv_slabs[k][:],
                                start=(k == 0), stop=(k == KT - 1))
                        for h8 in range(8):
                            hh = nch * 8 + h8
                            nc.any.tensor_copy(
                                out=v_aug[:, mtok, hh * (D + 1):hh * (D + 1) + D],
                                in_=ps[:, h8 * D:(h8 + 1) * D])

            tap("vaug0", v_aug[:, 0, :])

            # ---- attention: per head pair ----
            att = big.tile([128, KT, TOK], bf16, tag="E")
            with tc.tile_pool(name="pexp", bufs=4) as pexp, \
                 tc.tile_pool(name="pnorm", bufs=3) as pnorm:
                for t in range(KT):
                    psA = pav.tile([D + 1, 512], f32, tag="av")
                    psB = pav.tile([D + 1, 512], f32, tag="av")
                    for kb in range(KT):
                        pss = psc.tile([128, 1024], f32, tag="sc")
                        nc.tensor.matmul(
                            pss[:, 0:512],
                            kTt[0:64, t, kb * 128:(kb + 1) * 128],
                            qT[0:64, t, :], start=True, stop=True)
                        nc.tensor.matmul(
                            pss[:, 512:1024],
                            kTt[64:128, t, kb * 128:(kb + 1) * 128],
                            qT[64:128, t, :], start=True, stop=True)
                        ex = pexp.tile([128, 1024], bf16, tag="exp")
                        nc.scalar.activation(out=ex, in_=pss[:],
                                             func=AF.Exp, scale=1.0 / np.sqrt(D))
                        hA, hB = 2 * t, 2 * t + 1
                        nc.tensor.matmul(
                            psA, v_aug[:, kb, hA * (D + 1):(hA + 1) * (D + 1)],
                            ex[:, 0:512],
                            start=(kb == 0), stop=(kb == KT - 1))
                        nc.tensor.matmul(
                            psB, v_aug[:, kb, hB * (D + 1):(hB + 1) * (D + 1)],
                            ex[:, 512:1024],
                            start=(kb == 0), stop=(kb == KT - 1))
                    # stage unnormalized att to SBUF immediately (frees the
                    # av psum fast) and run the recip chain off the psum path
                    rbs = []
                    us = []
                    for psX in (psA, psB):
                        rx = pnorm.tile([1, TOK], f32, tag="rx")
                        nc.scalar.activation(out=rx, in_=psX[D:D + 1, :], func=AF.Ln)
                        nc.scalar.activation(out=rx, in_=rx, func=AF.Exp, scale=-1.0)
                        rb = pnorm.tile([64, TOK], f32, tag="rb")
                        nc.gpsimd.partition_broadcast(rb[:], rx[:])
                        rbs.append(rb)
                        attu = pnorm.tile([64, TOK], f32, tag="attu")
                        nc.vector.tensor_copy(out=attu, in_=psX[0:D, :])
                        us.append(attu)
                    nc.vector.tensor_mul(out=att[0:64, t, :],
                                         in0=us[0][:], in1=rbs[0][:])
                    nc.vector.tensor_mul(out=att[64:128, t, :],
                                         in0=us[1][:], in1=rbs[1][:])
                    nc.scalar.activation(out=att[:, t, :],
                                         in_=att[:, t, :],
                                         func=AF.Identity, bias=bvT[:, t:t + 1])

            tap("att0", att[:, 0, :])
            tap("att7", att[:, 7, :])

            # ---- proj -> yT = (proj + b_proj) * (1 + gate_msa) ----
            yT = big.tile([128, KT, TOK], f32, tag="F")
            with tc.tile_pool(name="pwproj", bufs=2) as pwproj:
                for mt in range(KT):
                    wt = pwproj.tile([128, KT, 128], bf16, tag="wproj")
                    nc.sync.dma_start(out=wt, in_=dwproj.ap()[mt])
                    ps = pmm.tile([128, 512], f32, tag="mm")
                    for k in range(KT):
                        nc.tensor.matmul(ps, wt[:, k, :], att[:, k, :],
                                         start=(k == 0), stop=(k == KT - 1))
                    nc.vector.tensor_scalar(
                        out=yT[:, mt, :].bitcast(f32r), in0=ps[:],
                        scalar1=bprojT[:, mt:mt + 1], scalar2=g1p[:, mt:mt + 1],
                        op0=OP.add, op1=OP.mult)  # f32r-rounded for LN2 mean mm

            # ---- LN2 stats + modulate -> xm2T ----
            pstat2_cm = tc.tile_pool(name="pstat2", bufs=1)
            pstat2 = pstat2_cm.__enter__()
            st2_mu = pstat2.tile([1, TOK], f32)
            st2_e2 = pstat2.tile([1, TOK], f32)
            st2_var = pstat2.tile([1, TOK], f32)
            st2_rstd = pstat2.tile([1, TOK], f32)
            with tc.tile_pool(name="pysq", bufs=2) as pysq:
                ps = pmm.tile([128, 512], f32, tag="mm")
                for k in range(KT):
                    yb = pysq.tile([128, TOK], bf16, tag="yb")
                    nc.vector.tensor_copy(out=yb, in_=yT[:, k, :])
                    nc.tensor.matmul(ps[0:1, :], ones_bf[:], yb[:],
                                     tile_position=(0, 0),
                                     start=(k == 0), stop=(k == KT - 1))
                for k in range(KT):
                    ysq = pysq.tile([128, TOK], bf16, tag="ysq")
                    nc.vector.tensor_mul(out=ysq, in0=yT[:, k, :], in1=yT[:, k, :])
                    nc.tensor.matmul(ps[32:33, :], ones_bf[:], ysq[:],
                                     tile_position=(0, 32),
                                     start=(k == 0), stop=(k == KT - 1))
                nc.scalar.activation(out=st2_mu[:].bitcast(f32r), in_=ps[0:1, :],
                                     func=AF.Copy, scale=1.0 / H)
                nc.scalar.activation(out=st2_e2, in_=ps[32:33, :], func=AF.Copy,
                                     scale=1.0 / H)
            nc.vector.tensor_mul(out=st2_var, in0=st2_mu, in1=st2_mu)
            nc.vector.tensor_sub(out=st2_var, in0=st2_e2, in1=st2_var)
            nc.scalar.activation(out=st2_var, in_=st2_var, func=AF.Ln, bias=eps_sb[:])
            nc.scalar.activation(out=st2_rstd[:].bitcast(f32r), in_=st2_var,
                                 func=AF.Exp, scale=-0.5)
            muB2 = psc.tile([128, TOK], f32, tag="sc", name="muB2")
            rstdB2 = psc.tile([128, TOK], f32, tag="sc", name="rstdB2")
            nc.tensor.matmul(muB2[:], ones_row[:].bitcast(f32r),
                             st2_mu[0:1, :].bitcast(f32r), start=True, stop=True)
            nc.tensor.matmul(rstdB2[:], ones_row[:].bitcast(f32r),
                             st2_rstd[0:1, :].bitcast(f32r), start=True, stop=True)
            pstat2_cm.__exit__(None, None, None)

            xm2T = big.tile([128, KT, TOK], bf16, tag="D")
            with tc.tile_pool(name="ptmp2", bufs=4) as ptmp2:
                for k in range(KT):
                    tmp = ptmp2.tile([128, TOK], f32, tag="tmp2")
                    nc.vector.tensor_sub(out=tmp, in0=yT[:, k, :], in1=muB2[:])
                    nc.vector.tensor_mul(out=tmp, in0=tmp, in1=rstdB2[:])
                    nc.scalar.activation(out=xm2T[:, k, :], in_=tmp,
                                         func=AF.Identity,
                                         scale=sc2p[:, k:k + 1],
                                         bias=cmT[:, 24 + k:24 + k + 1])

            tap("yT0", yT[:, 0, :])
            tap("xm2T0", xm2T[:, 0, :])

            # ---- mlp1: hidT = gelu(w1T @ xm2T + b1) (bf16) ----
            hidT = big.tile([128, MT_FF, TOK], bf16, tag="B")
            with tc.tile_pool(name="pw1", bufs=3) as pw1:
                for mt in range(MT_FF):
                    wt = pw1.tile([128, KT, 128], bf16, tag="w1")
                    nc.sync.dma_start(out=wt, in_=dw1.ap()[mt])
                    ps = pmm.tile([128, 512], f32, tag="mm")
                    for k in range(KT):
                        nc.tensor.matmul(ps, wt[:, k, :], xm2T[:, k, :],
                                         start=(k == 0), stop=(k == KT - 1))
                    nc.scalar.activation(out=hidT[:, mt, :], in_=ps[:],
                                         func=AF.Gelu, bias=b1T[:, mt:mt + 1])

            tap("hid0", hidT[:, 0, :])

            # ---- mlp2 + residual -> out ----
            with tc.tile_pool(name="pw2", bufs=2) as pw2, \
                 tc.tile_pool(name="pout", bufs=3) as pout:
                for mt in range(KT):
                    wt = pw2.tile([128, MT_FF, 128], bf16, tag="w2")
                    nc.sync.dma_start(out=wt, in_=dw2.ap()[mt])
                    ps = pmm.tile([128, 512], f32, tag="mm")
                    for kk in range(MT_FF):
                        nc.tensor.matmul(ps, wt[:, kk, :], hidT[:, kk, :],
                                         start=(kk == 0), stop=(kk == MT_FF - 1))
                    ot = pout.tile([128, TOK], f32, tag="out")
                    nc.vector.tensor_scalar(
                        out=ot, in0=ps[:], scalar1=b2T[:, mt:mt + 1],
                        scalar2=cmT[:, 40 + mt:40 + mt + 1],
                        op0=OP.add, op1=OP.mult)
                    nc.vector.tensor_add(out=ot, in0=ot, in1=yT[:, mt, :])
                    nc.sync.dma_start(out=doutT.ap()[mt * 128:(mt + 1) * 128, :],
                                      in_=ot)

    nc.compile()
    return nc


def _prep_shared(w_mod, b_mod, w_qkv, b_qkv, w_proj, b_proj, w1, b1, w2, b2):
    c32 = np.ascontiguousarray
    f32 = np.float32
    shared = {
        "wmod": c32(w_mod.reshape(8, 128, 12, 512).transpose(2, 1, 0, 3)).astype(ml_dtypes.bfloat16),
        "bmodT": c32(b_mod.reshape(48, 128).T).astype(f32, copy=False),
        "wqk": c32(w_qkv[:, :2048].reshape(8, 128, 16, 128).transpose(2, 1, 0, 3)).astype(ml_dtypes.bfloat16),
        "bqkT": c32(b_qkv[:2048].reshape(16, 128).T).astype(f32, copy=False),
        "wv": c32(w_qkv[:, 2048:]).astype(ml_dtypes.bfloat16),
        "bvT": c32(b_qkv[2048:].reshape(8, 128).T).astype(f32, copy=False),
        "wproj": c32(w_proj.reshape(8, 128, 8, 128).transpose(2, 1, 0, 3)).astype(ml_dtypes.bfloat16),
        "bprojT": c32(b_proj.reshape(8, 128).T).astype(f32, copy=False),
        "w1": c32(w1.reshape(8, 128, 32, 128).transpose(2, 1, 0, 3)).astype(ml_dtypes.bfloat16),
        "b1T": c32(b1.reshape(32, 128).T).astype(f32, copy=False),
        "w2": c32(w2.reshape(32, 128, 8, 128).transpose(2, 1, 0, 3)).astype(ml_dtypes.bfloat16),
        "b2T": c32(b2.reshape(8, 128).T).astype(f32, copy=False),
    }
    return shared


def make_in_maps(x, c, w_mod, b_mod, w_qkv, b_qkv, w_proj, b_proj, w1, b1, w2, b2):
    x = np.asarray(x, np.float32)
    c = np.asarray(c, np.float32)
    shared = _prep_shared(np.asarray(w_mod, np.float32), np.asarray(b_mod, np.float32),
                          np.asarray(w_qkv, np.float32), np.asarray(b_qkv, np.float32),
                          np.asarray(w_proj, np.float32), np.asarray(b_proj, np.float32),
                          np.asarray(w1, np.float32), np.asarray(b1, np.float32),
                          np.asarray(w2, np.float32), np.asarray(b2, np.float32))
    in_maps = []
    for ci in range(NCORES):
        b, h = divmod(ci, 2)
        xTb = x[b].T
        xcore = np.ascontiguousarray(
            np.concatenate([xTb[:, h * TOK:(h + 1) * TOK],
                            xTb[:, (1 - h) * TOK:(2 - h) * TOK]], axis=1))
        cTb = np.ascontiguousarray(c[b].reshape(8, 128).T)
        m = dict(shared)
        m["xT"] = xcore
        m["cT"] = cTb
        in_maps.append(m)
    return in_maps


def gather(results):
    out = np.empty((B, N, H), np.float32)
    for ci in range(NCORES):
        b, h = divmod(ci, 2)
        out[b, h * TOK:(h + 1) * TOK, :] = results[ci]["outT"].T
    return out


def get_nc(taps=()):
    key = ("nc", tuple(sorted(taps)))
    if key not in _CACHE:
        _CACHE[key] = _build(taps=taps)
    return _CACHE[key]


def kernel(**inputs):
    from concourse import bass_utils
    nc = get_nc()
    in_maps = make_in_maps(**inputs)
    res = bass_utils.run_bass_kernel_spmd(nc, in_maps,
                                          core_ids=list(range(NCORES)))
    return gather(res.results)
